# revision 11
# baseline (speedup 1.0000x reference)
"""Cross-WindowAttention Trainium2 kernel.

Full inputs -> shard batch dim over 8 NeuronCores -> bass/Tile kernel per core
-> gather. Host-side numpy does layout prep (transposes to feature-major,
bf16 conversion, combined rpb+mask bias table); the Bass kernel does all
matmul/softmax compute.

Per-core pipeline (shard = 256 windows of 64 tokens, 16384 rows):
 - qkv projections on PE in bf16, contraction over concat(x,y) for k/v.
   q,k produced feature-major [feat, rows]; v row-major per window [64, 512].
 - attention per (head-pair, 8-window chunk) in one [128, 512] PSUM bank,
   computed TRANSPOSED (S^T[m, n], stationary = k) so the softmax weights
   come out m-major and feed the PV matmul directly -- no PE transposes.
 - softmax normalization is deferred past PV: unnormalized U = exp @ v and
   den = sum_m exp (ones-stationary matmul into the same PSUM bank), then
   U * (1/den) is fused into the PSUM->SBUF staging multiply.  1/den is
   partition-replicated via GpSimd partition_broadcast (idle engine).
 - output projection with attention-output tiles stationary -> row-major
   result, biases via ones-row matmul, contiguous DMA out.

The chunk loop is software-pipelined by one chunk: the small attention/proj
matmul groups of chunk c-1 are emitted interleaved between the large qkv
matmul groups of chunk c, keeping the PE array duty cycle high enough that
the HAM activity monitor does not clock-gate it to half speed.
"""
import numpy as np
import ml_dtypes

import concourse.bacc as bacc
import concourse.mybir as mybir
from concourse.tile import TileContext
from concourse.bass_utils import run_bass_kernel_spmd

F32 = mybir.dt.float32
BF16 = mybir.dt.bfloat16
BF = ml_dtypes.bfloat16

N_CORES = 8
B_FULL = 2048
N = 64                      # window size (tokens per window)
C = 512                     # channels
H = 16                      # heads
HD = 32                     # head dim
CX = 512                    # x feature dim
CY = 1000                   # y feature dim
CYP = 1024                  # y feature dim padded to multiple of 128
SCALE = HD ** -0.5
C0 = 0.0                    # exp shift: exp(S - C0), cancels in U/den

B_SHARD = B_FULL // N_CORES             # 256 windows per core
WIN_PER_CHUNK = 8
ROWS_PER_CHUNK = WIN_PER_CHUNK * N      # 512
N_CHUNKS = B_SHARD // WIN_PER_CHUNK     # 32

KT_X = CX // 128            # 4 contraction tiles from x
KT_Y = CYP // 128           # 8 contraction tiles from y (padded)
FT_Q = C // 128             # 4 feature tiles per projection output


def build_nc(n_chunks=N_CHUNKS):
    rows = n_chunks * ROWS_PER_CHUNK
    nc = bacc.Bacc("TRN2", target_bir_lowering=False)

    xt = nc.dram_tensor("xt", [CX, rows], BF16, kind="ExternalInput")
    yt = nc.dram_tensor("yt", [CYP, rows], BF16, kind="ExternalInput")
    w1 = nc.dram_tensor("w1", [CX, 3 * C], BF16, kind="ExternalInput")
    w2 = nc.dram_tensor("w2", [CYP, 3 * C], BF16, kind="ExternalInput")
    wp = nc.dram_tensor("wp", [4, 128, C], BF16, kind="ExternalInput")  # quad-permuted rows
    cb = nc.dram_tensor("cb", [8, 8, 128, 512], BF16, kind="ExternalInput")
    bq = nc.dram_tensor("bq", [128, FT_Q], F32, kind="ExternalInput")
    bp = nc.dram_tensor("bp", [128, C], F32, kind="ExternalInput")
    out = nc.dram_tensor("out", [rows, C], F32, kind="ExternalOutput")

    with TileContext(nc) as tc:
        with tc.tile_pool(name="const", bufs=1) as constp, \
             tc.tile_pool(name="wpool", bufs=1) as wpool, \
             tc.tile_pool(name="stream", bufs=2) as stream, \
             tc.tile_pool(name="acts", bufs=2) as acts, \
             tc.tile_pool(name="small", bufs=3) as small, \
             tc.tile_pool(name="pbig", bufs=2, space="PSUM") as pbig, \
             tc.tile_pool(name="pattn", bufs=2, space="PSUM") as pattn, \
             tc.tile_pool(name="ppv", bufs=2, space="PSUM") as ppv:

            # ---- resident constants / weights
            w1_sb = wpool.tile([128, KT_X, 3 * C], BF16)
            nc.sync.dma_start(out=w1_sb, in_=w1.rearrange("(a p) f -> p a f", p=128))
            w2_sb = wpool.tile([128, KT_Y, 3 * C], BF16)
            nc.sync.dma_start(out=w2_sb, in_=w2.rearrange("(a p) f -> p a f", p=128))
            wp_sb = wpool.tile([128, 4, C], BF16)
            nc.sync.dma_start(out=wp_sb, in_=wp.rearrange("a p f -> p a f"))
            bq_sb = constp.tile([128, FT_Q], F32)
            nc.sync.dma_start(out=bq_sb, in_=bq[:, :])
            bp_sb = constp.tile([128, C], F32)
            nc.sync.dma_start(out=bp_sb, in_=bp[:, :])
            # den-matmul stationary: cols 0-31 sum partitions 0-63 (hh=0),
            # cols 32-63 sum partitions 64-127 (hh=1) -- denominator comes
            # out of the PE already replicated across the 32 d-partitions
            ones2_sb = constp.tile([128, 64], BF16)
            nc.vector.memset(ones2_sb[:, :], 0.0)
            nc.vector.memset(ones2_sb[0:64, 0:32], 1.0)
            nc.vector.memset(ones2_sb[64:128, 32:64], 1.0)

            xt_r = xt.rearrange("(a p) r -> p a r", p=128)
            yt_r = yt.rearrange("(a p) r -> p a r", p=128)

            st = {}  # per-chunk live tiles

            def emit_dma(c):
                r0 = c * ROWS_PER_CHUNK
                s = {}
                s["xt"] = stream.tile([128, KT_X, ROWS_PER_CHUNK], BF16, tag="xt", name="xt")
                nc.sync.dma_start(out=s["xt"], in_=xt_r[:, :, r0:r0 + ROWS_PER_CHUNK])
                s["yt"] = stream.tile([128, KT_Y, ROWS_PER_CHUNK], BF16, tag="yt", name="yt")
                nc.sync.dma_start(out=s["yt"], in_=yt_r[:, :, r0:r0 + ROWS_PER_CHUNK])
                s["cb"] = stream.tile([128, 8, 512], BF16, tag="cb", name="cbt")
                nc.sync.dma_start(out=s["cb"],
                                  in_=cb[c % 8].rearrange("hp p f -> p hp f"))
                s["q"] = acts.tile([128, FT_Q, ROWS_PER_CHUNK], BF16, tag="q", name="qsb")
                s["k"] = acts.tile([128, FT_Q, ROWS_PER_CHUNK], BF16, tag="k", name="ksb")
                # v lives on both partition halves (dup'd by DMA) so PV's
                # stationary base matches the moving exp slice's base 64*hh
                s["v"] = acts.tile([128, WIN_PER_CHUNK, C], BF16, tag="v", name="vsb")
                s["expT"] = acts.tile([128, 8, 512], BF16, tag="expT", name="expT")
                s["ot"] = acts.tile([128, 4 * ROWS_PER_CHUNK], BF16, tag="ot", name="otsb")
                st[c] = s

            def emit_qkv_group(c, g):
                s = st[c]
                if g < FT_Q:                      # q projection, feature tile g
                    ft = g
                    bank = pbig.tile([128, ROWS_PER_CHUNK], F32, tag="pq")
                    for kt in range(KT_X):
                        nc.tensor.matmul(
                            bank[:, :],
                            w1_sb[:, kt, 128 * ft:128 * (ft + 1)],
                            s["xt"][:, kt, :],
                            start=(kt == 0), stop=(kt == KT_X - 1))
                    nc.scalar.activation(
                        s["q"][:, ft, :], bank[:, :],
                        mybir.ActivationFunctionType.Identity,
                        bias=bq_sb[:, ft:ft + 1])
                elif g < 2 * FT_Q:                # k projection, feature tile g-4
                    ft = g - FT_Q
                    bank = pbig.tile([128, ROWS_PER_CHUNK], F32, tag="pq")
                    for kt in range(KT_X):
                        nc.tensor.matmul(
                            bank[:, :],
                            w1_sb[:, kt, C + 128 * ft:C + 128 * (ft + 1)],
                            s["xt"][:, kt, :],
                            start=(kt == 0), stop=False)
                    for kt in range(KT_Y):
                        nc.tensor.matmul(
                            bank[:, :],
                            w2_sb[:, kt, C + 128 * ft:C + 128 * (ft + 1)],
                            s["yt"][:, kt, :],
                            start=False, stop=(kt == KT_Y - 1))
                    nc.scalar.copy(s["k"][:, ft, :], bank[:, :])
                else:                             # v projection, row tile g-8
                    rt = g - 2 * FT_Q
                    bank = pbig.tile([128, C], F32, tag="pq")
                    for kt in range(KT_X):
                        nc.tensor.matmul(
                            bank[:, :],
                            s["xt"][:, kt, 128 * rt:128 * (rt + 1)],
                            w1_sb[:, kt, 2 * C:3 * C],
                            start=(kt == 0), stop=False)
                    for kt in range(KT_Y):
                        nc.tensor.matmul(
                            bank[:, :],
                            s["yt"][:, kt, 128 * rt:128 * (rt + 1)],
                            w2_sb[:, kt, 2 * C:3 * C],
                            start=False, stop=(kt == KT_Y - 1))
                    nc.scalar.copy(s["v"][0:64, 2 * rt, :], bank[0:64, :])
                    nc.vector.tensor_copy(s["v"][0:64, 2 * rt + 1, :], bank[64:128, :])
                    if rt == FT_Q - 1:
                        # replicate v into partitions 64-127 for hh=1 PV tiles
                        nc.sync.dma_start(out=s["v"][64:128, :, :],
                                          in_=s["v"][0:64, :, :])

            def emit_attn_group(c, hp):
                s = st[c]
                bank = pattn.tile([128, 512], F32, tag="pattn")
                # S^T[m, n] per (window, head): stationary = k, moving = q
                for sw in range(WIN_PER_CHUNK):
                    for hh in range(2):
                        h = 2 * hp + hh
                        pq = 32 * (h % 4)
                        ft = h // 4
                        nc.tensor.matmul(
                            bank[64 * hh:64 * (hh + 1), 64 * sw:64 * (sw + 1)],
                            s["k"][pq:pq + 32, ft, 64 * sw:64 * (sw + 1)],
                            s["q"][pq:pq + 32, ft, 64 * sw:64 * (sw + 1)],
                            start=True, stop=True, skip_group_check=True,
                            tile_position=(pq, 64 * hh))
                # combined rpb+mask bias (pre-divided by SCALE, incl -C0 shift)
                nc.vector.tensor_tensor(out=bank[:, :], in0=bank[:, :],
                                        in1=s["cb"][:, hp, :],
                                        op=mybir.AluOpType.add)
                expT = s["expT"]
                nc.scalar.activation(
                    expT[:, hp, :], bank[:, :],
                    mybir.ActivationFunctionType.Exp, scale=SCALE)
                pot = ppv.tile([128, 512], F32, tag="pot")
                # den rows 64-127: den[hh] replicated over the 32 d-partitions
                nc.tensor.matmul(
                    pot[64:128, :], ones2_sb[:, :], expT[:, hp, :],
                    start=True, stop=True, skip_group_check=True)
                # PV: U[d, n] = sum_m v[m, d] expT[m, n] (unnormalized)
                for sw in range(WIN_PER_CHUNK):
                    for hh in range(2):
                        h = 2 * hp + hh
                        nc.tensor.matmul(
                            pot[32 * hh:32 * (hh + 1), 64 * sw:64 * (sw + 1)],
                            s["v"][64 * hh:64 * (hh + 1), sw, HD * h:HD * (h + 1)],
                            expT[64 * hh:64 * (hh + 1), hp, 64 * sw:64 * (sw + 1)],
                            start=True, stop=True, skip_group_check=True,
                            tile_position=(64 * hh, 32 * hh))
                rrep = small.tile([64, 512], F32, tag="rrep")
                nc.vector.reciprocal(rrep[:, :], pot[64:128, :])
                # stage to SBUF with fused normalization:
                # ot[p=32*(h%4)+d, (t, q=h//4, w, n)] = U * (1/den)
                dst = s["ot"][64 * (hp % 2):64 * (hp % 2) + 64, :] \
                    .rearrange("p (t q w m) -> p t q w m", t=4, q=4, w=2) \
                    [:, :, hp // 2, :, :]
                nc.vector.tensor_tensor(
                    out=dst,
                    in0=pot[0:64, :].rearrange("p (t w m) -> p t w m", t=4, w=2),
                    in1=rrep.rearrange("p (t w m) -> p t w m", t=4, w=2),
                    op=mybir.AluOpType.mult)

            def emit_proj_group(c, rt):
                s = st[c]
                r0 = c * ROWS_PER_CHUNK
                bank = pbig.tile([128, C], F32, tag="pq")
                for quad in range(4):
                    nc.tensor.matmul(
                        bank[:, :],
                        s["ot"].rearrange("p (t q f) -> p t q f", t=4, q=4)
                        [:, rt, quad, :],
                        wp_sb[:, quad, :],
                        start=(quad == 0), stop=(quad == 3))
                out_f32 = small.tile([128, C], F32, tag="outf")
                nc.vector.tensor_tensor(out=out_f32[:, :], in0=bank[:, :],
                                        in1=bp_sb[:, :], op=mybir.AluOpType.add)
                nc.sync.dma_start(
                    out=out[r0 + 128 * rt:r0 + 128 * (rt + 1), :],
                    in_=out_f32[:, :])

            # software pipeline: big qkv groups of chunk c interleaved with
            # small attention/proj groups of chunk c-1
            for c in range(n_chunks + 1):
                if c < n_chunks:
                    emit_dma(c)
                big = [("qkv", c, g) for g in range(12)] if c < n_chunks else []
                smalls = ([("attn", c - 1, hp) for hp in range(8)]
                          + [("proj", c - 1, rt) for rt in range(4)]) if c > 0 else []
                order = []
                for i in range(max(len(big), len(smalls))):
                    if i < len(big):
                        order.append(big[i])
                    if i < len(smalls):
                        order.append(smalls[i])
                for kind, cc, idx in order:
                    if kind == "qkv":
                        emit_qkv_group(cc, idx)
                    elif kind == "attn":
                        emit_attn_group(cc, idx)
                    else:
                        emit_proj_group(cc, idx)
                if c > 0:
                    del st[c - 1]
    nc.compile()
    return nc


_NC_CACHE = {}


def _get_nc(n_chunks):
    if n_chunks not in _NC_CACHE:
        _NC_CACHE[n_chunks] = build_nc(n_chunks)
    return _NC_CACHE[n_chunks]


def prep_shared(w_qkv1, b_qkv1, w_qkv2, b_qkv2, bias_table, rel_index, w_proj,
                b_proj, mask):
    """Host-side prep of weights/bias tables shared by all cores."""
    w1 = w_qkv1.astype(BF)
    w2 = np.zeros((CYP, 3 * C), np.float32)
    w2[:CY] = w_qkv2
    # k/v biases ride an all-ones row in the padded region of yT
    w2[CY, C:2 * C] = b_qkv1[C:2 * C] + b_qkv2[C:2 * C]
    w2[CY, 2 * C:] = b_qkv1[2 * C:] + b_qkv2[2 * C:]
    w2 = w2.astype(BF)
    # quad-permuted rows: wp[Q, p, :] = w_proj[32*(4Q + p//32) + p%32, :]
    wp = np.empty((4, 128, C), np.float32)
    for q in range(4):
        for g in range(4):
            wp[q, 32 * g:32 * (g + 1), :] = \
                w_proj[32 * (4 * q + g):32 * (4 * q + g) + 32, :]
    wp = wp.astype(BF)

    bq = b_qkv1[0:C].reshape(FT_Q, 128).T.astype(np.float32).copy()
    bp = np.broadcast_to(b_proj.astype(np.float32), (128, C)).copy()

    rpb = bias_table[rel_index.reshape(-1)].reshape(N, N, H).transpose(2, 0, 1)
    cbt = (rpb[None] + mask[:, None] - C0) / SCALE      # [w, h, n, m]
    cb6 = cbt.reshape(8, 8, 8, 2, N, N)                 # [c8, sw, hp, hh, n, m]
    # transposed bank layout: [c8, hp, (hh, m), (sw, n)]
    cbd = np.ascontiguousarray(cb6.transpose(0, 2, 3, 5, 1, 4)) \
        .reshape(8, 8, 128, 512).astype(BF)

    return dict(w1=w1, w2=w2, wp=wp, bq=bq, bp=bp, cb=cbd)


def prep_core_inputs(x, y, shared, n_cores=N_CORES):
    """Split x, y along batch, transpose to feature-major, bf16."""
    B_, n, _ = x.shape
    rows = (B_ // n_cores) * n
    in_maps = []
    for i in range(n_cores):
        lo = i * (B_ // n_cores)
        hi = lo + B_ // n_cores
        xs = x[lo:hi].reshape(rows, CX)
        ys = y[lo:hi].reshape(rows, CY)
        xtb = np.ascontiguousarray(xs.T).astype(BF)
        ytb = np.zeros((CYP, rows), BF)
        ytb[:CY] = np.ascontiguousarray(ys.T).astype(BF)
        ytb[CY] = 1.0
        in_maps.append(dict(xt=xtb, yt=ytb, **shared))
    return in_maps


def kernel(x, y, mask, w_qkv1, b_qkv1, w_qkv2, b_qkv2, bias_table, rel_index,
           w_proj, b_proj, _n_cores=N_CORES, _trace=False):
    B_, n, _ = x.shape
    n_chunks = (B_ // _n_cores) // WIN_PER_CHUNK
    shared = prep_shared(np.asarray(w_qkv1), np.asarray(b_qkv1),
                         np.asarray(w_qkv2), np.asarray(b_qkv2),
                         np.asarray(bias_table), np.asarray(rel_index),
                         np.asarray(w_proj), np.asarray(b_proj),
                         np.asarray(mask))
    in_maps = prep_core_inputs(np.asarray(x), np.asarray(y), shared, _n_cores)
    nc = _get_nc(n_chunks)
    res = run_bass_kernel_spmd(nc, in_maps, core_ids=list(range(_n_cores)),
                               trace=_trace)
    outs = [res.results[i]["out"].reshape(B_ // _n_cores, n, C)
            for i in range(_n_cores)]
    full = np.concatenate(outs, axis=0)
    kernel.last_results = res
    return full


# revision 15
# speedup vs baseline: 1.2912x; 1.2912x over previous
"""Cross-WindowAttention Trainium2 kernel.

Full inputs -> shard batch dim over 8 NeuronCores -> bass/Tile kernel per core
-> gather. Host-side numpy does layout prep (transposes to feature-major,
bf16 conversion, combined rpb+mask bias table); the Bass kernel does all
matmul/softmax compute.

Per-core pipeline (shard = 256 windows of 64 tokens, 16384 rows):
 - qkv projections on PE in bf16, contraction over concat(x,y) for k/v.
   q,k produced feature-major [feat, rows]; v row-major per window [64, 512].
 - attention per (head-pair, 8-window chunk) in one [128, 512] PSUM bank,
   computed TRANSPOSED (S^T[m, n], stationary = k) so the softmax weights
   come out m-major and feed the PV matmul directly -- no PE transposes.
 - softmax normalization is deferred past PV: unnormalized U = exp @ v and
   den = sum_m exp (ones-stationary matmul into the same PSUM bank), then
   U * (1/den) is fused into the PSUM->SBUF staging multiply.  1/den is
   partition-replicated via GpSimd partition_broadcast (idle engine).
 - output projection with attention-output tiles stationary -> row-major
   result, biases via ones-row matmul, contiguous DMA out.

The chunk loop is software-pipelined by one chunk: the small attention/proj
matmul groups of chunk c-1 are emitted interleaved between the large qkv
matmul groups of chunk c, keeping the PE array duty cycle high enough that
the HAM activity monitor does not clock-gate it to half speed.
"""
import numpy as np
import ml_dtypes

import concourse.bacc as bacc
import concourse.mybir as mybir
from concourse.tile import TileContext
from concourse.bass_utils import run_bass_kernel_spmd

F32 = mybir.dt.float32
BF16 = mybir.dt.bfloat16
BF = ml_dtypes.bfloat16

N_CORES = 8
B_FULL = 2048
N = 64                      # window size (tokens per window)
C = 512                     # channels
H = 16                      # heads
HD = 32                     # head dim
CX = 512                    # x feature dim
CY = 1000                   # y feature dim
CYP = 1024                  # y feature dim padded to multiple of 128
SCALE = HD ** -0.5
C0 = 0.0                    # exp shift: exp(S - C0), cancels in U/den

B_SHARD = B_FULL // N_CORES             # 256 windows per core
WIN_PER_CHUNK = 8
ROWS_PER_CHUNK = WIN_PER_CHUNK * N      # 512
N_CHUNKS = B_SHARD // WIN_PER_CHUNK     # 32

KT_X = CX // 128            # 4 contraction tiles from x
KT_Y = CYP // 128           # 8 contraction tiles from y (padded)
FT_Q = C // 128             # 4 feature tiles per projection output


def build_nc(n_chunks=N_CHUNKS):
    rows = n_chunks * ROWS_PER_CHUNK
    nc = bacc.Bacc("TRN2", target_bir_lowering=False)

    xt = nc.dram_tensor("xt", [CX, rows], BF16, kind="ExternalInput")
    yt = nc.dram_tensor("yt", [CYP, rows], BF16, kind="ExternalInput")
    w1 = nc.dram_tensor("w1", [CX, 3 * C], BF16, kind="ExternalInput")
    w2 = nc.dram_tensor("w2", [CYP, 3 * C], BF16, kind="ExternalInput")
    wp = nc.dram_tensor("wp", [4, 128, C], BF16, kind="ExternalInput")  # quad-permuted rows
    cb = nc.dram_tensor("cb", [8, 8, 128, 512], BF16, kind="ExternalInput")
    bq = nc.dram_tensor("bq", [128, FT_Q], F32, kind="ExternalInput")
    bp = nc.dram_tensor("bp", [128, C], F32, kind="ExternalInput")
    out = nc.dram_tensor("out", [rows, C], F32, kind="ExternalOutput")

    with TileContext(nc) as tc:
        with tc.tile_pool(name="const", bufs=1) as constp, \
             tc.tile_pool(name="wpool", bufs=1) as wpool, \
             tc.tile_pool(name="stream", bufs=2) as stream, \
             tc.tile_pool(name="acts", bufs=2) as acts, \
             tc.tile_pool(name="small", bufs=3) as small, \
             tc.tile_pool(name="pbig", bufs=2, space="PSUM") as pbig, \
             tc.tile_pool(name="pattn", bufs=2, space="PSUM") as pattn, \
             tc.tile_pool(name="ppv", bufs=2, space="PSUM") as ppv, \
             tc.tile_pool(name="pden", bufs=2, space="PSUM") as pden:

            # ---- resident constants / weights
            w1_sb = wpool.tile([128, KT_X, 3 * C], BF16)
            nc.sync.dma_start(out=w1_sb, in_=w1.rearrange("(a p) f -> p a f", p=128))
            w2_sb = wpool.tile([128, KT_Y, 3 * C], BF16)
            nc.sync.dma_start(out=w2_sb, in_=w2.rearrange("(a p) f -> p a f", p=128))
            wp_sb = wpool.tile([128, 4, C], BF16)
            nc.sync.dma_start(out=wp_sb, in_=wp.rearrange("a p f -> p a f"))
            bq_sb = constp.tile([128, FT_Q], F32)
            nc.sync.dma_start(out=bq_sb, in_=bq[:, :])
            bp_sb = constp.tile([128, C], F32)
            nc.sync.dma_start(out=bp_sb, in_=bp[:, :])
            # den-matmul stationary: cols 0-31 sum partitions 0-63 (hh=0),
            # cols 32-63 sum partitions 64-127 (hh=1) -- denominator comes
            # out of the PE already replicated across the 32 d-partitions
            ones2_sb = constp.tile([128, 64], BF16)
            nc.vector.memset(ones2_sb[:, :], 0.0)
            nc.vector.memset(ones2_sb[0:64, 0:32], 1.0)
            nc.vector.memset(ones2_sb[64:128, 32:64], 1.0)

            xt_r = xt.rearrange("(a p) r -> p a r", p=128)
            yt_r = yt.rearrange("(a p) r -> p a r", p=128)

            st = {}  # per-chunk live tiles

            def emit_dma(c):
                r0 = c * ROWS_PER_CHUNK
                s = {}
                s["xt"] = stream.tile([128, KT_X, ROWS_PER_CHUNK], BF16, tag="xt", name="xt")
                nc.sync.dma_start(out=s["xt"], in_=xt_r[:, :, r0:r0 + ROWS_PER_CHUNK])
                s["yt"] = stream.tile([128, KT_Y, ROWS_PER_CHUNK], BF16, tag="yt", name="yt")
                nc.sync.dma_start(out=s["yt"], in_=yt_r[:, :, r0:r0 + ROWS_PER_CHUNK])
                s["cb"] = stream.tile([128, 8, 512], BF16, tag="cb", name="cbt")
                nc.sync.dma_start(out=s["cb"],
                                  in_=cb[c % 8].rearrange("hp p f -> p hp f"))
                s["q"] = acts.tile([128, FT_Q, ROWS_PER_CHUNK], BF16, tag="q", name="qsb")
                s["k"] = acts.tile([128, FT_Q, ROWS_PER_CHUNK], BF16, tag="k", name="ksb")
                # v lives on both partition halves (dup'd by DMA) so PV's
                # stationary base matches the moving exp slice's base 64*hh
                s["v"] = acts.tile([128, WIN_PER_CHUNK, C], BF16, tag="v", name="vsb")
                s["expT"] = acts.tile([128, 8, 512], BF16, tag="expT", name="expT")
                s["ot"] = acts.tile([128, 4 * ROWS_PER_CHUNK], BF16, tag="ot", name="otsb")
                st[c] = s

            def emit_qkv_group(c, g):
                s = st[c]
                if g < FT_Q:                      # q projection, feature tile g
                    ft = g
                    bank = pbig.tile([128, ROWS_PER_CHUNK], F32, tag="pq")
                    for kt in range(KT_X):
                        nc.tensor.matmul(
                            bank[:, :],
                            w1_sb[:, kt, 128 * ft:128 * (ft + 1)],
                            s["xt"][:, kt, :],
                            start=(kt == 0), stop=(kt == KT_X - 1))
                    nc.scalar.activation(
                        s["q"][:, ft, :], bank[:, :],
                        mybir.ActivationFunctionType.Identity,
                        bias=bq_sb[:, ft:ft + 1])
                elif g < 2 * FT_Q:                # k projection, feature tile g-4
                    ft = g - FT_Q
                    bank = pbig.tile([128, ROWS_PER_CHUNK], F32, tag="pq")
                    for kt in range(KT_X):
                        nc.tensor.matmul(
                            bank[:, :],
                            w1_sb[:, kt, C + 128 * ft:C + 128 * (ft + 1)],
                            s["xt"][:, kt, :],
                            start=(kt == 0), stop=False)
                    for kt in range(KT_Y):
                        nc.tensor.matmul(
                            bank[:, :],
                            w2_sb[:, kt, C + 128 * ft:C + 128 * (ft + 1)],
                            s["yt"][:, kt, :],
                            start=False, stop=(kt == KT_Y - 1))
                    nc.scalar.copy(s["k"][:, ft, :], bank[:, :])
                else:                             # v projection, row tile g-8
                    rt = g - 2 * FT_Q
                    bank = pbig.tile([128, C], F32, tag="pq")
                    for kt in range(KT_X):
                        nc.tensor.matmul(
                            bank[:, :],
                            s["xt"][:, kt, 128 * rt:128 * (rt + 1)],
                            w1_sb[:, kt, 2 * C:3 * C],
                            start=(kt == 0), stop=False)
                    for kt in range(KT_Y):
                        nc.tensor.matmul(
                            bank[:, :],
                            s["yt"][:, kt, 128 * rt:128 * (rt + 1)],
                            w2_sb[:, kt, 2 * C:3 * C],
                            start=False, stop=(kt == KT_Y - 1))
                    nc.scalar.copy(s["v"][0:64, 2 * rt, :], bank[0:64, :])
                    nc.vector.tensor_copy(s["v"][0:64, 2 * rt + 1, :], bank[64:128, :])
                    if rt == FT_Q - 1:
                        # replicate v into partitions 64-127 for hh=1 PV tiles
                        nc.sync.dma_start(out=s["v"][64:128, :, :],
                                          in_=s["v"][0:64, :, :])

            def emit_attn_group(c, hp):
                s = st[c]
                bank = pattn.tile([128, 512], F32, tag="pattn")
                # S^T[m, n] per (window, head): stationary = k, moving = q
                for sw in range(WIN_PER_CHUNK):
                    for hh in range(2):
                        h = 2 * hp + hh
                        pq = 32 * (h % 4)
                        ft = h // 4
                        nc.tensor.matmul(
                            bank[64 * hh:64 * (hh + 1), 64 * sw:64 * (sw + 1)],
                            s["k"][pq:pq + 32, ft, 64 * sw:64 * (sw + 1)],
                            s["q"][pq:pq + 32, ft, 64 * sw:64 * (sw + 1)],
                            start=True, stop=True, skip_group_check=True,
                            tile_position=(pq, 64 * hh))
                # combined rpb+mask bias (pre-divided by SCALE, incl -C0 shift)
                nc.vector.tensor_tensor(out=bank[:, :], in0=bank[:, :],
                                        in1=s["cb"][:, hp, :],
                                        op=mybir.AluOpType.add)
                expT = s["expT"]
                nc.scalar.activation(
                    expT[:, hp, :], bank[:, :],
                    mybir.ActivationFunctionType.Exp, scale=SCALE)
                pot = ppv.tile([64, 512], F32, tag="pot")
                # den bank (base partition 0 -- reciprocal_approx_fast and
                # other custom ops misread partition-offset inputs on HW):
                # den[hh] replicated over the 32 d-partitions
                dbank = pden.tile([64, 512], F32, tag="pden")
                nc.tensor.matmul(
                    dbank[:, :], ones2_sb[:, :], expT[:, hp, :],
                    start=True, stop=True, skip_group_check=True)
                # PV: U[d, n] = sum_m v[m, d] expT[m, n] (unnormalized)
                for sw in range(WIN_PER_CHUNK):
                    for hh in range(2):
                        h = 2 * hp + hh
                        nc.tensor.matmul(
                            pot[32 * hh:32 * (hh + 1), 64 * sw:64 * (sw + 1)],
                            s["v"][64 * hh:64 * (hh + 1), sw, HD * h:HD * (h + 1)],
                            expT[64 * hh:64 * (hh + 1), hp, 64 * sw:64 * (sw + 1)],
                            start=True, stop=True, skip_group_check=True,
                            tile_position=(64 * hh, 32 * hh))
                rrep = small.tile([64, 512], F32, tag="rrep")
                nc.vector.reciprocal_approx_fast(rrep[:, :], dbank[:, :])
                # stage to SBUF with fused normalization:
                # ot[p=32*(h%4)+d, (t, q=h//4, w, n)] = U * (1/den)
                dst = s["ot"][64 * (hp % 2):64 * (hp % 2) + 64, :] \
                    .rearrange("p (t q w m) -> p t q w m", t=4, q=4, w=2) \
                    [:, :, hp // 2, :, :]
                nc.vector.tensor_tensor(
                    out=dst,
                    in0=pot[0:64, :].rearrange("p (t w m) -> p t w m", t=4, w=2),
                    in1=rrep.rearrange("p (t w m) -> p t w m", t=4, w=2),
                    op=mybir.AluOpType.mult)

            def emit_proj_group(c, rt):
                s = st[c]
                r0 = c * ROWS_PER_CHUNK
                bank = pbig.tile([128, C], F32, tag="pq")
                for quad in range(4):
                    nc.tensor.matmul(
                        bank[:, :],
                        s["ot"].rearrange("p (t q f) -> p t q f", t=4, q=4)
                        [:, rt, quad, :],
                        wp_sb[:, quad, :],
                        start=(quad == 0), stop=(quad == 3))
                out_f32 = small.tile([128, C], F32, tag="outf")
                nc.vector.tensor_tensor(out=out_f32[:, :], in0=bank[:, :],
                                        in1=bp_sb[:, :], op=mybir.AluOpType.add)
                nc.sync.dma_start(
                    out=out[r0 + 128 * rt:r0 + 128 * (rt + 1), :],
                    in_=out_f32[:, :])

            # software pipeline: big qkv groups of chunk c interleaved with
            # small attention/proj groups of chunk c-1
            for c in range(n_chunks + 1):
                if c < n_chunks:
                    emit_dma(c)
                big = [("qkv", c, g) for g in range(12)] if c < n_chunks else []
                smalls = ([("attn", c - 1, hp) for hp in range(8)]
                          + [("proj", c - 1, rt) for rt in range(4)]) if c > 0 else []
                order = []
                for i in range(max(len(big), len(smalls))):
                    if i < len(big):
                        order.append(big[i])
                    if i < len(smalls):
                        order.append(smalls[i])
                for kind, cc, idx in order:
                    if kind == "qkv":
                        emit_qkv_group(cc, idx)
                    elif kind == "attn":
                        emit_attn_group(cc, idx)
                    else:
                        emit_proj_group(cc, idx)
                if c > 0:
                    del st[c - 1]
    nc.compile()
    return nc


_NC_CACHE = {}


def _get_nc(n_chunks):
    if n_chunks not in _NC_CACHE:
        _NC_CACHE[n_chunks] = build_nc(n_chunks)
    return _NC_CACHE[n_chunks]


def prep_shared(w_qkv1, b_qkv1, w_qkv2, b_qkv2, bias_table, rel_index, w_proj,
                b_proj, mask):
    """Host-side prep of weights/bias tables shared by all cores."""
    w1 = w_qkv1.astype(BF)
    w2 = np.zeros((CYP, 3 * C), np.float32)
    w2[:CY] = w_qkv2
    # k/v biases ride an all-ones row in the padded region of yT
    w2[CY, C:2 * C] = b_qkv1[C:2 * C] + b_qkv2[C:2 * C]
    w2[CY, 2 * C:] = b_qkv1[2 * C:] + b_qkv2[2 * C:]
    w2 = w2.astype(BF)
    # quad-permuted rows: wp[Q, p, :] = w_proj[32*(4Q + p//32) + p%32, :]
    wp = np.empty((4, 128, C), np.float32)
    for q in range(4):
        for g in range(4):
            wp[q, 32 * g:32 * (g + 1), :] = \
                w_proj[32 * (4 * q + g):32 * (4 * q + g) + 32, :]
    wp = wp.astype(BF)

    bq = b_qkv1[0:C].reshape(FT_Q, 128).T.astype(np.float32).copy()
    bp = np.broadcast_to(b_proj.astype(np.float32), (128, C)).copy()

    rpb = bias_table[rel_index.reshape(-1)].reshape(N, N, H).transpose(2, 0, 1)
    cbt = (rpb[None] + mask[:, None] - C0) / SCALE      # [w, h, n, m]
    cb6 = cbt.reshape(8, 8, 8, 2, N, N)                 # [c8, sw, hp, hh, n, m]
    # transposed bank layout: [c8, hp, (hh, m), (sw, n)]
    cbd = np.ascontiguousarray(cb6.transpose(0, 2, 3, 5, 1, 4)) \
        .reshape(8, 8, 128, 512).astype(BF)

    return dict(w1=w1, w2=w2, wp=wp, bq=bq, bp=bp, cb=cbd)


def prep_core_inputs(x, y, shared, n_cores=N_CORES):
    """Split x, y along batch, transpose to feature-major, bf16."""
    B_, n, _ = x.shape
    rows = (B_ // n_cores) * n
    in_maps = []
    for i in range(n_cores):
        lo = i * (B_ // n_cores)
        hi = lo + B_ // n_cores
        xs = x[lo:hi].reshape(rows, CX)
        ys = y[lo:hi].reshape(rows, CY)
        xtb = np.ascontiguousarray(xs.T).astype(BF)
        ytb = np.zeros((CYP, rows), BF)
        ytb[:CY] = np.ascontiguousarray(ys.T).astype(BF)
        ytb[CY] = 1.0
        in_maps.append(dict(xt=xtb, yt=ytb, **shared))
    return in_maps


def kernel(x, y, mask, w_qkv1, b_qkv1, w_qkv2, b_qkv2, bias_table, rel_index,
           w_proj, b_proj, _n_cores=N_CORES, _trace=False):
    B_, n, _ = x.shape
    n_chunks = (B_ // _n_cores) // WIN_PER_CHUNK
    shared = prep_shared(np.asarray(w_qkv1), np.asarray(b_qkv1),
                         np.asarray(w_qkv2), np.asarray(b_qkv2),
                         np.asarray(bias_table), np.asarray(rel_index),
                         np.asarray(w_proj), np.asarray(b_proj),
                         np.asarray(mask))
    in_maps = prep_core_inputs(np.asarray(x), np.asarray(y), shared, _n_cores)
    nc = _get_nc(n_chunks)
    res = run_bass_kernel_spmd(nc, in_maps, core_ids=list(range(_n_cores)),
                               trace=_trace)
    outs = [res.results[i]["out"].reshape(B_ // _n_cores, n, C)
            for i in range(_n_cores)]
    full = np.concatenate(outs, axis=0)
    kernel.last_results = res
    return full


# revision 22
# speedup vs baseline: 1.3608x; 1.0539x over previous
"""Cross-WindowAttention Trainium2 kernel.

Full inputs -> shard batch dim over 8 NeuronCores -> bass/Tile kernel per core
-> gather. Host-side numpy does layout prep (transposes to feature-major,
bf16 conversion, combined rpb+mask bias table); the Bass kernel does all
matmul/softmax compute.

Per-core pipeline (shard = 256 windows of 64 tokens, 16384 rows):
 - qkv projections on PE in bf16, contraction over concat(x,y) for k/v.
   q,k produced feature-major [feat, rows]; v row-major per window [64, 512].
 - attention per (head-pair, 8-window chunk) in one [128, 512] PSUM bank,
   computed TRANSPOSED (S^T[m, n], stationary = k) so the softmax weights
   come out m-major and feed the PV matmul directly -- no PE transposes.
 - softmax normalization is deferred past PV: unnormalized U = exp @ v and
   den = sum_m exp (ones-stationary matmul into the same PSUM bank), then
   U * (1/den) is fused into the PSUM->SBUF staging multiply.  1/den is
   partition-replicated via GpSimd partition_broadcast (idle engine).
 - output projection with attention-output tiles stationary -> row-major
   result, biases via ones-row matmul, contiguous DMA out.

The chunk loop is software-pipelined by one chunk: the small attention/proj
matmul groups of chunk c-1 are emitted interleaved between the large qkv
matmul groups of chunk c, keeping the PE array duty cycle high enough that
the HAM activity monitor does not clock-gate it to half speed.
"""
import numpy as np
import ml_dtypes

import concourse.bacc as bacc
import concourse.mybir as mybir
from concourse.tile import TileContext
from concourse.bass_utils import run_bass_kernel_spmd

F32 = mybir.dt.float32
BF16 = mybir.dt.bfloat16
BF = ml_dtypes.bfloat16

N_CORES = 8
B_FULL = 2048
N = 64                      # window size (tokens per window)
C = 512                     # channels
H = 16                      # heads
HD = 32                     # head dim
CX = 512                    # x feature dim
CY = 1000                   # y feature dim
CYP = 1024                  # y feature dim padded to multiple of 128
SCALE = HD ** -0.5
C0 = 0.0                    # exp shift: exp(S - C0), cancels in U/den

B_SHARD = B_FULL // N_CORES             # 256 windows per core
WIN_PER_CHUNK = 8
ROWS_PER_CHUNK = WIN_PER_CHUNK * N      # 512
N_CHUNKS = B_SHARD // WIN_PER_CHUNK     # 32

KT_X = CX // 128            # 4 contraction tiles from x
KT_Y = CYP // 128           # 8 contraction tiles from y (padded)
FT_Q = C // 128             # 4 feature tiles per projection output


def build_nc(n_chunks=N_CHUNKS):
    rows = n_chunks * ROWS_PER_CHUNK
    nc = bacc.Bacc("TRN2", target_bir_lowering=False)

    xt = nc.dram_tensor("xt", [CX, rows], BF16, kind="ExternalInput")
    yt = nc.dram_tensor("yt", [CYP, rows], BF16, kind="ExternalInput")
    w1 = nc.dram_tensor("w1", [CX, 3 * C], BF16, kind="ExternalInput")
    w2 = nc.dram_tensor("w2", [CYP, 3 * C], BF16, kind="ExternalInput")
    wp = nc.dram_tensor("wp", [4, 128, C], BF16, kind="ExternalInput")  # quad-permuted rows
    cb = nc.dram_tensor("cb", [8, 8, 128, 512], BF16, kind="ExternalInput")
    bq = nc.dram_tensor("bq", [128, FT_Q], F32, kind="ExternalInput")
    bp = nc.dram_tensor("bp", [128, C], F32, kind="ExternalInput")
    ident = nc.dram_tensor("ident", [128, 128], BF16, kind="ExternalInput")
    out = nc.dram_tensor("out", [rows, C], F32, kind="ExternalOutput")

    with TileContext(nc) as tc:
        with tc.tile_pool(name="const", bufs=1) as constp, \
             tc.tile_pool(name="wpool", bufs=1) as wpool, \
             tc.tile_pool(name="stream", bufs=2) as stream, \
             tc.tile_pool(name="acts", bufs=2) as acts, \
             tc.tile_pool(name="small", bufs=3) as small, \
             tc.tile_pool(name="pbig", bufs=2, space="PSUM") as pbig, \
             tc.tile_pool(name="pattn", bufs=2, space="PSUM") as pattn, \
             tc.tile_pool(name="ppv", bufs=2, space="PSUM") as ppv, \
             tc.tile_pool(name="pden", bufs=2, space="PSUM") as pden:

            # ---- resident constants / weights
            w1_sb = wpool.tile([128, KT_X, 3 * C], BF16)
            nc.sync.dma_start(out=w1_sb, in_=w1.rearrange("(a p) f -> p a f", p=128))
            w2_sb = wpool.tile([128, KT_Y, 3 * C], BF16)
            nc.sync.dma_start(out=w2_sb, in_=w2.rearrange("(a p) f -> p a f", p=128))
            wp_sb = wpool.tile([128, 4, C], BF16)
            nc.sync.dma_start(out=wp_sb, in_=wp.rearrange("a p f -> p a f"))
            bq_sb = constp.tile([128, FT_Q], F32)
            nc.sync.dma_start(out=bq_sb, in_=bq[:, :])
            bp_sb = constp.tile([128, C], F32)
            nc.sync.dma_start(out=bp_sb, in_=bp[:, :])
            # den-matmul stationary: cols 0-31 sum partitions 0-63 (hh=0),
            # cols 32-63 sum partitions 64-127 (hh=1) -- denominator comes
            # out of the PE already replicated across the 32 d-partitions
            ones2_sb = constp.tile([128, 64], BF16)
            nc.vector.memset(ones2_sb[:, :], 0.0)
            nc.vector.memset(ones2_sb[0:64, 0:32], 1.0)
            nc.vector.memset(ones2_sb[64:128, 32:64], 1.0)
            id_sb = constp.tile([128, 128], BF16)
            nc.sync.dma_start(out=id_sb, in_=ident[:, :])

            xt_r = xt.rearrange("(a p) r -> p a r", p=128)
            yt_r = yt.rearrange("(a p) r -> p a r", p=128)

            st = {}  # per-chunk live tiles

            def emit_dma(c):
                r0 = c * ROWS_PER_CHUNK
                s = {}
                s["xt"] = stream.tile([128, KT_X, ROWS_PER_CHUNK], BF16, tag="xt", name="xt")
                nc.sync.dma_start(out=s["xt"], in_=xt_r[:, :, r0:r0 + ROWS_PER_CHUNK])
                s["yt"] = stream.tile([128, KT_Y, ROWS_PER_CHUNK], BF16, tag="yt", name="yt")
                nc.sync.dma_start(out=s["yt"], in_=yt_r[:, :, r0:r0 + ROWS_PER_CHUNK])
                s["cb"] = stream.tile([128, 8, 512], BF16, tag="cb", name="cbt")
                nc.sync.dma_start(out=s["cb"],
                                  in_=cb[c % 8].rearrange("hp p f -> p hp f"))
                s["q"] = acts.tile([128, FT_Q, ROWS_PER_CHUNK], BF16, tag="q", name="qsb")
                s["k"] = acts.tile([128, FT_Q, ROWS_PER_CHUNK], BF16, tag="k", name="ksb")
                # v lives on both partition halves (dup'd by DMA) so PV's
                # stationary base matches the moving exp slice's base 64*hh
                s["v"] = acts.tile([128, WIN_PER_CHUNK, C], BF16, tag="v", name="vsb")
                s["expT"] = acts.tile([128, 8, 512], BF16, tag="expT", name="expT")
                s["ot"] = acts.tile([128, 4 * ROWS_PER_CHUNK], BF16, tag="ot", name="otsb")
                st[c] = s

            def emit_qkv_group(c, g):
                s = st[c]
                if g < FT_Q:                      # q projection, feature tile g
                    ft = g
                    bank = pbig.tile([128, ROWS_PER_CHUNK], F32, tag="pq")
                    for kt in range(KT_X):
                        nc.tensor.matmul(
                            bank[:, :],
                            w1_sb[:, kt, 128 * ft:128 * (ft + 1)],
                            s["xt"][:, kt, :],
                            start=(kt == 0), stop=(kt == KT_X - 1))
                    nc.scalar.activation(
                        s["q"][:, ft, :], bank[:, :],
                        mybir.ActivationFunctionType.Identity,
                        bias=bq_sb[:, ft:ft + 1])
                elif g < 2 * FT_Q:                # k projection, feature tile g-4
                    ft = g - FT_Q
                    bank = pbig.tile([128, ROWS_PER_CHUNK], F32, tag="pq")
                    for kt in range(KT_X):
                        nc.tensor.matmul(
                            bank[:, :],
                            w1_sb[:, kt, C + 128 * ft:C + 128 * (ft + 1)],
                            s["xt"][:, kt, :],
                            start=(kt == 0), stop=False)
                    for kt in range(KT_Y):
                        nc.tensor.matmul(
                            bank[:, :],
                            w2_sb[:, kt, C + 128 * ft:C + 128 * (ft + 1)],
                            s["yt"][:, kt, :],
                            start=False, stop=(kt == KT_Y - 1))
                    nc.scalar.copy(s["k"][:, ft, :], bank[:, :])
                else:                             # v projection, row tile g-8
                    rt = g - 2 * FT_Q
                    bank = pbig.tile([128, C], F32, tag="pq")
                    for kt in range(KT_X):
                        nc.tensor.matmul(
                            bank[:, :],
                            s["xt"][:, kt, 128 * rt:128 * (rt + 1)],
                            w1_sb[:, kt, 2 * C:3 * C],
                            start=(kt == 0), stop=False)
                    for kt in range(KT_Y):
                        nc.tensor.matmul(
                            bank[:, :],
                            s["yt"][:, kt, 128 * rt:128 * (rt + 1)],
                            w2_sb[:, kt, 2 * C:3 * C],
                            start=False, stop=(kt == KT_Y - 1))
                    nc.scalar.copy(s["v"][0:64, 2 * rt, :], bank[0:64, :])
                    nc.vector.tensor_copy(s["v"][0:64, 2 * rt + 1, :], bank[64:128, :])
                    if rt == FT_Q - 1:
                        # replicate v into partitions 64-127 for hh=1 PV tiles
                        nc.sync.dma_start(out=s["v"][64:128, :, :],
                                          in_=s["v"][0:64, :, :])

            def emit_attn_a(c, hp):
                # QK + bias + exp: the only PE->other-engine handoff; the
                # consuming den/PV matmuls are emitted several slots later
                # (emit_attn_b) so the PE FIFO never stalls on exp.
                s = st[c]
                bank = pattn.tile([128, 512], F32, tag="pattn")
                # combined rpb+mask bias (pre-divided by SCALE, incl -C0
                # shift) written first via identity matmul; the QK matmuls
                # then accumulate onto it (PE-only, no DVE in the chain)
                nc.tensor.matmul(
                    bank[:, :], id_sb[:, :], s["cb"][:, hp, :],
                    start=True, stop=False, skip_group_check=True)
                # S^T[m, n] per (window, head): stationary = k, moving = q
                for sw in range(WIN_PER_CHUNK):
                    for hh in range(2):
                        h = 2 * hp + hh
                        pq = 32 * (h % 4)
                        ft = h // 4
                        nc.tensor.matmul(
                            bank[64 * hh:64 * (hh + 1), 64 * sw:64 * (sw + 1)],
                            s["k"][pq:pq + 32, ft, 64 * sw:64 * (sw + 1)],
                            s["q"][pq:pq + 32, ft, 64 * sw:64 * (sw + 1)],
                            start=False,
                            stop=(sw == WIN_PER_CHUNK - 1 and hh == 1),
                            skip_group_check=True,
                            tile_position=(pq, 64 * hh))
                expT = s["expT"]
                nc.scalar.activation(
                    expT[:, hp, :], bank[:, :],
                    mybir.ActivationFunctionType.Exp, scale=SCALE)

            def emit_attn_b(c, hp):
                s = st[c]
                expT = s["expT"]
                pot = ppv.tile([64, 512], F32, tag="pot")
                # den bank (base partition 0 -- reciprocal_approx_fast and
                # other custom ops misread partition-offset inputs on HW):
                # den[hh] replicated over the 32 d-partitions
                dbank = pden.tile([64, 512], F32, tag="pden")
                nc.tensor.matmul(
                    dbank[:, :], ones2_sb[:, :], expT[:, hp, :],
                    start=True, stop=True, skip_group_check=True)
                # PV: U[d, n] = sum_m v[m, d] expT[m, n] (unnormalized)
                for sw in range(WIN_PER_CHUNK):
                    for hh in range(2):
                        h = 2 * hp + hh
                        nc.tensor.matmul(
                            pot[32 * hh:32 * (hh + 1), 64 * sw:64 * (sw + 1)],
                            s["v"][64 * hh:64 * (hh + 1), sw, HD * h:HD * (h + 1)],
                            expT[64 * hh:64 * (hh + 1), hp, 64 * sw:64 * (sw + 1)],
                            start=True, stop=True, skip_group_check=True,
                            tile_position=(64 * hh, 32 * hh))
                rrep = small.tile([64, 512], F32, tag="rrep")
                nc.vector.reciprocal_approx_fast(rrep[:, :], dbank[:, :])
                # stage to SBUF with fused normalization:
                # ot[p=32*(h%4)+d, (t, q=h//4, w, n)] = U * (1/den)
                dst = s["ot"][64 * (hp % 2):64 * (hp % 2) + 64, :] \
                    .rearrange("p (t q w m) -> p t q w m", t=4, q=4, w=2) \
                    [:, :, hp // 2, :, :]
                nc.vector.tensor_tensor(
                    out=dst,
                    in0=pot[0:64, :].rearrange("p (t w m) -> p t w m", t=4, w=2),
                    in1=rrep.rearrange("p (t w m) -> p t w m", t=4, w=2),
                    op=mybir.AluOpType.mult)

            def emit_proj_group(c, rt):
                s = st[c]
                r0 = c * ROWS_PER_CHUNK
                bank = pbig.tile([128, C], F32, tag="pq")
                for quad in range(4):
                    nc.tensor.matmul(
                        bank[:, :],
                        s["ot"].rearrange("p (t q f) -> p t q f", t=4, q=4)
                        [:, rt, quad, :],
                        wp_sb[:, quad, :],
                        start=(quad == 0), stop=(quad == 3))
                out_f32 = small.tile([128, C], F32, tag="outf")
                nc.vector.tensor_tensor(out=out_f32[:, :], in0=bank[:, :],
                                        in1=bp_sb[:, :], op=mybir.AluOpType.add)
                nc.sync.dma_start(
                    out=out[r0 + 128 * rt:r0 + 128 * (rt + 1), :],
                    in_=out_f32[:, :])

            # software pipeline: big qkv groups of chunk c interleaved with
            # small attention/proj groups of chunk c-1.  attn part B (den/PV,
            # needs exp of part A) trails part A by two slots so the PE FIFO
            # never waits on the ScalarE exp.
            for c in range(n_chunks + 1):
                if c < n_chunks:
                    emit_dma(c)
                big = [("qkv", c, g) for g in range(12)] if c < n_chunks else []
                if c > 0:
                    smalls = [("atta", c - 1, 0), ("atta", c - 1, 1)]
                    for hp in range(2, 8):
                        smalls += [("attb", c - 1, hp - 2), ("atta", c - 1, hp)]
                    smalls += [("attb", c - 1, 6), ("attb", c - 1, 7)]
                    smalls += [("proj", c - 1, rt) for rt in range(4)]
                else:
                    smalls = []
                order = []
                for i in range(max(len(big), len(smalls))):
                    if i < len(big):
                        order.append(big[i])
                    if i < len(smalls):
                        order.append(smalls[i])
                for kind, cc, idx in order:
                    if kind == "qkv":
                        emit_qkv_group(cc, idx)
                    elif kind == "atta":
                        emit_attn_a(cc, idx)
                    elif kind == "attb":
                        emit_attn_b(cc, idx)
                    else:
                        emit_proj_group(cc, idx)
                if c > 0:
                    del st[c - 1]
    nc.compile()
    return nc


_NC_CACHE = {}


def _get_nc(n_chunks):
    if n_chunks not in _NC_CACHE:
        _NC_CACHE[n_chunks] = build_nc(n_chunks)
    return _NC_CACHE[n_chunks]


def prep_shared(w_qkv1, b_qkv1, w_qkv2, b_qkv2, bias_table, rel_index, w_proj,
                b_proj, mask):
    """Host-side prep of weights/bias tables shared by all cores."""
    w1 = w_qkv1.astype(BF)
    w2 = np.zeros((CYP, 3 * C), np.float32)
    w2[:CY] = w_qkv2
    # k/v biases ride an all-ones row in the padded region of yT
    w2[CY, C:2 * C] = b_qkv1[C:2 * C] + b_qkv2[C:2 * C]
    w2[CY, 2 * C:] = b_qkv1[2 * C:] + b_qkv2[2 * C:]
    w2 = w2.astype(BF)
    # quad-permuted rows: wp[Q, p, :] = w_proj[32*(4Q + p//32) + p%32, :]
    wp = np.empty((4, 128, C), np.float32)
    for q in range(4):
        for g in range(4):
            wp[q, 32 * g:32 * (g + 1), :] = \
                w_proj[32 * (4 * q + g):32 * (4 * q + g) + 32, :]
    wp = wp.astype(BF)

    bq = b_qkv1[0:C].reshape(FT_Q, 128).T.astype(np.float32).copy()
    bp = np.broadcast_to(b_proj.astype(np.float32), (128, C)).copy()

    rpb = bias_table[rel_index.reshape(-1)].reshape(N, N, H).transpose(2, 0, 1)
    cbt = (rpb[None] + mask[:, None] - C0) / SCALE      # [w, h, n, m]
    cb6 = cbt.reshape(8, 8, 8, 2, N, N)                 # [c8, sw, hp, hh, n, m]
    # transposed bank layout: [c8, hp, (hh, m), (sw, n)]
    cbd = np.ascontiguousarray(cb6.transpose(0, 2, 3, 5, 1, 4)) \
        .reshape(8, 8, 128, 512).astype(BF)

    ident = np.eye(128, dtype=BF)
    return dict(w1=w1, w2=w2, wp=wp, bq=bq, bp=bp, cb=cbd, ident=ident)


def prep_core_inputs(x, y, shared, n_cores=N_CORES):
    """Split x, y along batch, transpose to feature-major, bf16."""
    B_, n, _ = x.shape
    rows = (B_ // n_cores) * n
    in_maps = []
    for i in range(n_cores):
        lo = i * (B_ // n_cores)
        hi = lo + B_ // n_cores
        xs = x[lo:hi].reshape(rows, CX)
        ys = y[lo:hi].reshape(rows, CY)
        xtb = np.ascontiguousarray(xs.T).astype(BF)
        ytb = np.zeros((CYP, rows), BF)
        ytb[:CY] = np.ascontiguousarray(ys.T).astype(BF)
        ytb[CY] = 1.0
        in_maps.append(dict(xt=xtb, yt=ytb, **shared))
    return in_maps


def kernel(x, y, mask, w_qkv1, b_qkv1, w_qkv2, b_qkv2, bias_table, rel_index,
           w_proj, b_proj, _n_cores=N_CORES, _trace=False):
    B_, n, _ = x.shape
    n_chunks = (B_ // _n_cores) // WIN_PER_CHUNK
    shared = prep_shared(np.asarray(w_qkv1), np.asarray(b_qkv1),
                         np.asarray(w_qkv2), np.asarray(b_qkv2),
                         np.asarray(bias_table), np.asarray(rel_index),
                         np.asarray(w_proj), np.asarray(b_proj),
                         np.asarray(mask))
    in_maps = prep_core_inputs(np.asarray(x), np.asarray(y), shared, _n_cores)
    nc = _get_nc(n_chunks)
    res = run_bass_kernel_spmd(nc, in_maps, core_ids=list(range(_n_cores)),
                               trace=_trace)
    outs = [res.results[i]["out"].reshape(B_ // _n_cores, n, C)
            for i in range(_n_cores)]
    full = np.concatenate(outs, axis=0)
    kernel.last_results = res
    return full


# revision 24
# speedup vs baseline: 1.3766x; 1.0116x over previous
"""Cross-WindowAttention Trainium2 kernel.

Full inputs -> shard batch dim over 8 NeuronCores -> bass/Tile kernel per core
-> gather. Host-side numpy does layout prep (transposes to feature-major,
bf16 conversion, combined rpb+mask bias table); the Bass kernel does all
matmul/softmax compute.

Per-core pipeline (shard = 256 windows of 64 tokens, 16384 rows):
 - qkv projections on PE in bf16, contraction over concat(x,y) for k/v.
   q,k produced feature-major [feat, rows]; v row-major per window [64, 512].
 - attention per (head-pair, 8-window chunk) in one [128, 512] PSUM bank,
   computed TRANSPOSED (S^T[m, n], stationary = k) so the softmax weights
   come out m-major and feed the PV matmul directly -- no PE transposes.
 - softmax normalization is deferred past PV: unnormalized U = exp @ v and
   den = sum_m exp (ones-stationary matmul into the same PSUM bank), then
   U * (1/den) is fused into the PSUM->SBUF staging multiply.  1/den is
   partition-replicated via GpSimd partition_broadcast (idle engine).
 - output projection with attention-output tiles stationary -> row-major
   result, biases via ones-row matmul, contiguous DMA out.

The chunk loop is software-pipelined by one chunk: the small attention/proj
matmul groups of chunk c-1 are emitted interleaved between the large qkv
matmul groups of chunk c, keeping the PE array duty cycle high enough that
the HAM activity monitor does not clock-gate it to half speed.
"""
import numpy as np
import ml_dtypes

import concourse.bacc as bacc
import concourse.mybir as mybir
from concourse.tile import TileContext
from concourse.bass_utils import run_bass_kernel_spmd

F32 = mybir.dt.float32
BF16 = mybir.dt.bfloat16
BF = ml_dtypes.bfloat16

N_CORES = 8
B_FULL = 2048
N = 64                      # window size (tokens per window)
C = 512                     # channels
H = 16                      # heads
HD = 32                     # head dim
CX = 512                    # x feature dim
CY = 1000                   # y feature dim
CYP = 1024                  # y feature dim padded to multiple of 128
SCALE = HD ** -0.5
C0 = 0.0                    # exp shift: exp(S - C0), cancels in U/den

B_SHARD = B_FULL // N_CORES             # 256 windows per core
WIN_PER_CHUNK = 8
ROWS_PER_CHUNK = WIN_PER_CHUNK * N      # 512
N_CHUNKS = B_SHARD // WIN_PER_CHUNK     # 32

KT_X = CX // 128            # 4 contraction tiles from x
KT_Y = CYP // 128           # 8 contraction tiles from y (padded)
FT_Q = C // 128             # 4 feature tiles per projection output


def build_nc(n_chunks=N_CHUNKS):
    rows = n_chunks * ROWS_PER_CHUNK
    nc = bacc.Bacc("TRN2", target_bir_lowering=False)

    xt = nc.dram_tensor("xt", [CX, rows], BF16, kind="ExternalInput")
    yt = nc.dram_tensor("yt", [CYP, rows], BF16, kind="ExternalInput")
    w1 = nc.dram_tensor("w1", [CX, 3 * C], BF16, kind="ExternalInput")
    w2 = nc.dram_tensor("w2", [CYP, 3 * C], BF16, kind="ExternalInput")
    wp = nc.dram_tensor("wp", [4, 128, C], BF16, kind="ExternalInput")  # quad-permuted rows
    cb = nc.dram_tensor("cb", [8, 8, 128, 512], BF16, kind="ExternalInput")
    bq = nc.dram_tensor("bq", [128, FT_Q], F32, kind="ExternalInput")
    bp = nc.dram_tensor("bp", [128, C], F32, kind="ExternalInput")
    ident = nc.dram_tensor("ident", [128, 128], BF16, kind="ExternalInput")
    out = nc.dram_tensor("out", [rows, C], F32, kind="ExternalOutput")

    with TileContext(nc) as tc:
        with tc.tile_pool(name="const", bufs=1) as constp, \
             tc.tile_pool(name="wpool", bufs=1) as wpool, \
             tc.tile_pool(name="stream", bufs=3) as stream, \
             tc.tile_pool(name="acts", bufs=2) as acts, \
             tc.tile_pool(name="small", bufs=3) as small, \
             tc.tile_pool(name="pbig", bufs=2, space="PSUM") as pbig, \
             tc.tile_pool(name="pattn", bufs=2, space="PSUM") as pattn, \
             tc.tile_pool(name="ppv", bufs=2, space="PSUM") as ppv, \
             tc.tile_pool(name="pden", bufs=2, space="PSUM") as pden:

            # ---- resident constants / weights
            w1_sb = wpool.tile([128, KT_X, 3 * C], BF16)
            nc.sync.dma_start(out=w1_sb, in_=w1.rearrange("(a p) f -> p a f", p=128))
            w2_sb = wpool.tile([128, KT_Y, 3 * C], BF16)
            nc.sync.dma_start(out=w2_sb, in_=w2.rearrange("(a p) f -> p a f", p=128))
            wp_sb = wpool.tile([128, 4, C], BF16)
            nc.sync.dma_start(out=wp_sb, in_=wp.rearrange("a p f -> p a f"))
            bq_sb = constp.tile([128, FT_Q], F32)
            nc.sync.dma_start(out=bq_sb, in_=bq[:, :])
            bp_sb = constp.tile([128, C], F32)
            nc.sync.dma_start(out=bp_sb, in_=bp[:, :])
            # den-matmul stationary: cols 0-31 sum partitions 0-63 (hh=0),
            # cols 32-63 sum partitions 64-127 (hh=1) -- denominator comes
            # out of the PE already replicated across the 32 d-partitions
            ones2_sb = constp.tile([128, 64], BF16)
            nc.vector.memset(ones2_sb[:, :], 0.0)
            nc.vector.memset(ones2_sb[0:64, 0:32], 1.0)
            nc.vector.memset(ones2_sb[64:128, 32:64], 1.0)
            id_sb = constp.tile([128, 128], BF16)
            nc.sync.dma_start(out=id_sb, in_=ident[:, :])

            xt_r = xt.rearrange("(a p) r -> p a r", p=128)
            yt_r = yt.rearrange("(a p) r -> p a r", p=128)

            st = {}  # per-chunk live tiles

            def emit_dma(c):
                r0 = c * ROWS_PER_CHUNK
                s = {}
                s["xt"] = stream.tile([128, KT_X, ROWS_PER_CHUNK], BF16, tag="xt", name="xt")
                nc.sync.dma_start(out=s["xt"], in_=xt_r[:, :, r0:r0 + ROWS_PER_CHUNK])
                s["yt"] = stream.tile([128, KT_Y, ROWS_PER_CHUNK], BF16, tag="yt", name="yt")
                nc.sync.dma_start(out=s["yt"], in_=yt_r[:, :, r0:r0 + ROWS_PER_CHUNK])
                s["cb"] = stream.tile([128, 8, 512], BF16, tag="cb", name="cbt")
                nc.sync.dma_start(out=s["cb"],
                                  in_=cb[c % 8].rearrange("hp p f -> p hp f"))
                s["q"] = acts.tile([128, FT_Q, ROWS_PER_CHUNK], BF16, tag="q", name="qsb")
                s["k"] = acts.tile([128, FT_Q, ROWS_PER_CHUNK], BF16, tag="k", name="ksb")
                # v lives on both partition halves (dup'd by DMA) so PV's
                # stationary base matches the moving exp slice's base 64*hh
                s["v"] = acts.tile([128, WIN_PER_CHUNK, C], BF16, tag="v", name="vsb")
                s["expT"] = acts.tile([128, 8, 512], BF16, tag="expT", name="expT")
                s["ot"] = acts.tile([128, 4 * ROWS_PER_CHUNK], BF16, tag="ot", name="otsb")
                st[c] = s

            def emit_qkv_group(c, g):
                s = st[c]
                if g < FT_Q:                      # q projection, feature tile g
                    ft = g
                    bank = pbig.tile([128, ROWS_PER_CHUNK], F32, tag="pq")
                    for kt in range(KT_X):
                        nc.tensor.matmul(
                            bank[:, :],
                            w1_sb[:, kt, 128 * ft:128 * (ft + 1)],
                            s["xt"][:, kt, :],
                            start=(kt == 0), stop=(kt == KT_X - 1))
                    nc.scalar.activation(
                        s["q"][:, ft, :], bank[:, :],
                        mybir.ActivationFunctionType.Identity,
                        bias=bq_sb[:, ft:ft + 1])
                elif g < 2 * FT_Q:                # k projection, feature tile g-4
                    ft = g - FT_Q
                    bank = pbig.tile([128, ROWS_PER_CHUNK], F32, tag="pq")
                    for kt in range(KT_X):
                        nc.tensor.matmul(
                            bank[:, :],
                            w1_sb[:, kt, C + 128 * ft:C + 128 * (ft + 1)],
                            s["xt"][:, kt, :],
                            start=(kt == 0), stop=False)
                    for kt in range(KT_Y):
                        nc.tensor.matmul(
                            bank[:, :],
                            w2_sb[:, kt, C + 128 * ft:C + 128 * (ft + 1)],
                            s["yt"][:, kt, :],
                            start=False, stop=(kt == KT_Y - 1))
                    nc.scalar.copy(s["k"][:, ft, :], bank[:, :])
                else:                             # v projection, row tile g-8
                    rt = g - 2 * FT_Q
                    bank = pbig.tile([128, C], F32, tag="pq")
                    for kt in range(KT_X):
                        nc.tensor.matmul(
                            bank[:, :],
                            s["xt"][:, kt, 128 * rt:128 * (rt + 1)],
                            w1_sb[:, kt, 2 * C:3 * C],
                            start=(kt == 0), stop=False)
                    for kt in range(KT_Y):
                        nc.tensor.matmul(
                            bank[:, :],
                            s["yt"][:, kt, 128 * rt:128 * (rt + 1)],
                            w2_sb[:, kt, 2 * C:3 * C],
                            start=False, stop=(kt == KT_Y - 1))
                    nc.scalar.copy(s["v"][0:64, 2 * rt, :], bank[0:64, :])
                    nc.vector.tensor_copy(s["v"][0:64, 2 * rt + 1, :], bank[64:128, :])
                    if rt == FT_Q - 1:
                        # replicate v into partitions 64-127 for hh=1 PV tiles
                        nc.sync.dma_start(out=s["v"][64:128, :, :],
                                          in_=s["v"][0:64, :, :])

            def emit_attn_a(c, hp):
                # QK + bias + exp: the only PE->other-engine handoff; the
                # consuming den/PV matmuls are emitted several slots later
                # (emit_attn_b) so the PE FIFO never stalls on exp.
                s = st[c]
                bank = pattn.tile([128, 512], F32, tag="pattn")
                # combined rpb+mask bias (pre-divided by SCALE, incl -C0
                # shift) written first via identity matmul; the QK matmuls
                # then accumulate onto it (PE-only, no DVE in the chain)
                nc.tensor.matmul(
                    bank[:, :], id_sb[:, :], s["cb"][:, hp, :],
                    start=True, stop=False, skip_group_check=True)
                # S^T[m, n] per (window, head): stationary = k, moving = q
                for sw in range(WIN_PER_CHUNK):
                    for hh in range(2):
                        h = 2 * hp + hh
                        pq = 32 * (h % 4)
                        ft = h // 4
                        nc.tensor.matmul(
                            bank[64 * hh:64 * (hh + 1), 64 * sw:64 * (sw + 1)],
                            s["k"][pq:pq + 32, ft, 64 * sw:64 * (sw + 1)],
                            s["q"][pq:pq + 32, ft, 64 * sw:64 * (sw + 1)],
                            start=False,
                            stop=(sw == WIN_PER_CHUNK - 1 and hh == 1),
                            skip_group_check=True,
                            tile_position=(pq, 64 * hh))
                expT = s["expT"]
                nc.scalar.activation(
                    expT[:, hp, :], bank[:, :],
                    mybir.ActivationFunctionType.Exp, scale=SCALE)

            def emit_attn_b(c, hp):
                s = st[c]
                expT = s["expT"]
                pot = ppv.tile([64, 512], F32, tag="pot")
                # den bank (base partition 0 -- reciprocal_approx_fast and
                # other custom ops misread partition-offset inputs on HW):
                # den[hh] replicated over the 32 d-partitions
                dbank = pden.tile([64, 512], F32, tag="pden")
                nc.tensor.matmul(
                    dbank[:, :], ones2_sb[:, :], expT[:, hp, :],
                    start=True, stop=True, skip_group_check=True)
                # PV: U[d, n] = sum_m v[m, d] expT[m, n] (unnormalized)
                for sw in range(WIN_PER_CHUNK):
                    for hh in range(2):
                        h = 2 * hp + hh
                        nc.tensor.matmul(
                            pot[32 * hh:32 * (hh + 1), 64 * sw:64 * (sw + 1)],
                            s["v"][64 * hh:64 * (hh + 1), sw, HD * h:HD * (h + 1)],
                            expT[64 * hh:64 * (hh + 1), hp, 64 * sw:64 * (sw + 1)],
                            start=True, stop=True, skip_group_check=True,
                            tile_position=(64 * hh, 32 * hh))
                rrep = small.tile([64, 512], F32, tag="rrep")
                nc.vector.reciprocal_approx_fast(rrep[:, :], dbank[:, :])
                # stage to SBUF with fused normalization:
                # ot[p=32*(h%4)+d, (t, q=h//4, w, n)] = U * (1/den)
                dst = s["ot"][64 * (hp % 2):64 * (hp % 2) + 64, :] \
                    .rearrange("p (t q w m) -> p t q w m", t=4, q=4, w=2) \
                    [:, :, hp // 2, :, :]
                nc.vector.tensor_tensor(
                    out=dst,
                    in0=pot[0:64, :].rearrange("p (t w m) -> p t w m", t=4, w=2),
                    in1=rrep.rearrange("p (t w m) -> p t w m", t=4, w=2),
                    op=mybir.AluOpType.mult)

            def emit_proj_group(c, rt):
                s = st[c]
                r0 = c * ROWS_PER_CHUNK
                bank = pbig.tile([128, C], F32, tag="pq")
                for quad in range(4):
                    nc.tensor.matmul(
                        bank[:, :],
                        s["ot"].rearrange("p (t q f) -> p t q f", t=4, q=4)
                        [:, rt, quad, :],
                        wp_sb[:, quad, :],
                        start=(quad == 0), stop=(quad == 3))
                out_f32 = small.tile([128, C], F32, tag="outf")
                nc.vector.tensor_tensor(out=out_f32[:, :], in0=bank[:, :],
                                        in1=bp_sb[:, :], op=mybir.AluOpType.add)
                nc.sync.dma_start(
                    out=out[r0 + 128 * rt:r0 + 128 * (rt + 1), :],
                    in_=out_f32[:, :])

            # software pipeline: big qkv groups of chunk c interleaved with
            # small attention/proj groups of chunk c-1.  attn part B (den/PV,
            # needs exp of part A) trails part A by two slots so the PE FIFO
            # never waits on the ScalarE exp.
            emit_dma(0)
            for c in range(n_chunks + 1):
                # prefetch next chunk's inputs a full chunk ahead so the
                # chunk-boundary qkv matmuls never wait on DMA
                if c + 1 < n_chunks:
                    emit_dma(c + 1)
                big = [("qkv", c, g) for g in range(12)] if c < n_chunks else []
                if c > 0:
                    smalls = [("atta", c - 1, 0), ("atta", c - 1, 1)]
                    for hp in range(2, 8):
                        smalls += [("attb", c - 1, hp - 2), ("atta", c - 1, hp)]
                    smalls += [("attb", c - 1, 6), ("attb", c - 1, 7)]
                    smalls += [("proj", c - 1, rt) for rt in range(4)]
                else:
                    smalls = []
                order = []
                for i in range(max(len(big), len(smalls))):
                    if i < len(big):
                        order.append(big[i])
                    if i < len(smalls):
                        order.append(smalls[i])
                for kind, cc, idx in order:
                    if kind == "qkv":
                        emit_qkv_group(cc, idx)
                    elif kind == "atta":
                        emit_attn_a(cc, idx)
                    elif kind == "attb":
                        emit_attn_b(cc, idx)
                    else:
                        emit_proj_group(cc, idx)
                if c > 0:
                    del st[c - 1]
    nc.compile()
    return nc


_NC_CACHE = {}


def _get_nc(n_chunks):
    if n_chunks not in _NC_CACHE:
        _NC_CACHE[n_chunks] = build_nc(n_chunks)
    return _NC_CACHE[n_chunks]


def prep_shared(w_qkv1, b_qkv1, w_qkv2, b_qkv2, bias_table, rel_index, w_proj,
                b_proj, mask):
    """Host-side prep of weights/bias tables shared by all cores."""
    w1 = w_qkv1.astype(BF)
    w2 = np.zeros((CYP, 3 * C), np.float32)
    w2[:CY] = w_qkv2
    # k/v biases ride an all-ones row in the padded region of yT
    w2[CY, C:2 * C] = b_qkv1[C:2 * C] + b_qkv2[C:2 * C]
    w2[CY, 2 * C:] = b_qkv1[2 * C:] + b_qkv2[2 * C:]
    w2 = w2.astype(BF)
    # quad-permuted rows: wp[Q, p, :] = w_proj[32*(4Q + p//32) + p%32, :]
    wp = np.empty((4, 128, C), np.float32)
    for q in range(4):
        for g in range(4):
            wp[q, 32 * g:32 * (g + 1), :] = \
                w_proj[32 * (4 * q + g):32 * (4 * q + g) + 32, :]
    wp = wp.astype(BF)

    bq = b_qkv1[0:C].reshape(FT_Q, 128).T.astype(np.float32).copy()
    bp = np.broadcast_to(b_proj.astype(np.float32), (128, C)).copy()

    rpb = bias_table[rel_index.reshape(-1)].reshape(N, N, H).transpose(2, 0, 1)
    cbt = (rpb[None] + mask[:, None] - C0) / SCALE      # [w, h, n, m]
    cb6 = cbt.reshape(8, 8, 8, 2, N, N)                 # [c8, sw, hp, hh, n, m]
    # transposed bank layout: [c8, hp, (hh, m), (sw, n)]
    cbd = np.ascontiguousarray(cb6.transpose(0, 2, 3, 5, 1, 4)) \
        .reshape(8, 8, 128, 512).astype(BF)

    ident = np.eye(128, dtype=BF)
    return dict(w1=w1, w2=w2, wp=wp, bq=bq, bp=bp, cb=cbd, ident=ident)


def prep_core_inputs(x, y, shared, n_cores=N_CORES):
    """Split x, y along batch, transpose to feature-major, bf16."""
    B_, n, _ = x.shape
    rows = (B_ // n_cores) * n
    in_maps = []
    for i in range(n_cores):
        lo = i * (B_ // n_cores)
        hi = lo + B_ // n_cores
        xs = x[lo:hi].reshape(rows, CX)
        ys = y[lo:hi].reshape(rows, CY)
        xtb = np.ascontiguousarray(xs.T).astype(BF)
        ytb = np.zeros((CYP, rows), BF)
        ytb[:CY] = np.ascontiguousarray(ys.T).astype(BF)
        ytb[CY] = 1.0
        in_maps.append(dict(xt=xtb, yt=ytb, **shared))
    return in_maps


def kernel(x, y, mask, w_qkv1, b_qkv1, w_qkv2, b_qkv2, bias_table, rel_index,
           w_proj, b_proj, _n_cores=N_CORES, _trace=False):
    B_, n, _ = x.shape
    n_chunks = (B_ // _n_cores) // WIN_PER_CHUNK
    shared = prep_shared(np.asarray(w_qkv1), np.asarray(b_qkv1),
                         np.asarray(w_qkv2), np.asarray(b_qkv2),
                         np.asarray(bias_table), np.asarray(rel_index),
                         np.asarray(w_proj), np.asarray(b_proj),
                         np.asarray(mask))
    in_maps = prep_core_inputs(np.asarray(x), np.asarray(y), shared, _n_cores)
    nc = _get_nc(n_chunks)
    res = run_bass_kernel_spmd(nc, in_maps, core_ids=list(range(_n_cores)),
                               trace=_trace)
    outs = [res.results[i]["out"].reshape(B_ // _n_cores, n, C)
            for i in range(_n_cores)]
    full = np.concatenate(outs, axis=0)
    kernel.last_results = res
    return full


# revision 25
# speedup vs baseline: 1.4950x; 1.0860x over previous
"""Cross-WindowAttention Trainium2 kernel.

Full inputs -> shard batch dim over 8 NeuronCores -> bass/Tile kernel per core
-> gather. Host-side numpy does layout prep (transposes to feature-major,
bf16 conversion, combined rpb+mask bias table); the Bass kernel does all
matmul/softmax compute.

Per-core pipeline (shard = 256 windows of 64 tokens, 16384 rows):
 - qkv projections on PE in bf16, contraction over concat(x,y) for k/v.
   q,k produced feature-major [feat, rows]; v row-major per window [64, 512].
 - attention per (head-pair, 8-window chunk) in one [128, 512] PSUM bank,
   computed TRANSPOSED (S^T[m, n], stationary = k) so the softmax weights
   come out m-major and feed the PV matmul directly -- no PE transposes.
 - softmax normalization is deferred past PV: unnormalized U = exp @ v and
   den = sum_m exp (ones-stationary matmul into the same PSUM bank), then
   U * (1/den) is fused into the PSUM->SBUF staging multiply.  1/den is
   partition-replicated via GpSimd partition_broadcast (idle engine).
 - output projection with attention-output tiles stationary -> row-major
   result, biases via ones-row matmul, contiguous DMA out.

The chunk loop is software-pipelined by one chunk: the small attention/proj
matmul groups of chunk c-1 are emitted interleaved between the large qkv
matmul groups of chunk c, keeping the PE array duty cycle high enough that
the HAM activity monitor does not clock-gate it to half speed.
"""
import numpy as np
import ml_dtypes

import concourse.bacc as bacc
import concourse.mybir as mybir
from concourse.tile import TileContext
from concourse.bass_utils import run_bass_kernel_spmd

F32 = mybir.dt.float32
BF16 = mybir.dt.bfloat16
BF = ml_dtypes.bfloat16

N_CORES = 8
B_FULL = 2048
N = 64                      # window size (tokens per window)
C = 512                     # channels
H = 16                      # heads
HD = 32                     # head dim
CX = 512                    # x feature dim
CY = 1000                   # y feature dim
CYP = 1024                  # y feature dim padded to multiple of 128
SCALE = HD ** -0.5
C0 = 0.0                    # exp shift: exp(S - C0), cancels in U/den

B_SHARD = B_FULL // N_CORES             # 256 windows per core
WIN_PER_CHUNK = 8
ROWS_PER_CHUNK = WIN_PER_CHUNK * N      # 512
N_CHUNKS = B_SHARD // WIN_PER_CHUNK     # 32

KT_X = CX // 128            # 4 contraction tiles from x
KT_Y = CYP // 128           # 8 contraction tiles from y (padded)
FT_Q = C // 128             # 4 feature tiles per projection output


def build_nc(n_chunks=N_CHUNKS):
    rows = n_chunks * ROWS_PER_CHUNK
    nc = bacc.Bacc("TRN2", target_bir_lowering=False)

    xt = nc.dram_tensor("xt", [CX, rows], BF16, kind="ExternalInput")
    yt = nc.dram_tensor("yt", [CYP, rows], BF16, kind="ExternalInput")
    w1 = nc.dram_tensor("w1", [CX, 3 * C], BF16, kind="ExternalInput")
    w2 = nc.dram_tensor("w2", [CYP, 3 * C], BF16, kind="ExternalInput")
    wp = nc.dram_tensor("wp", [4, 128, C], BF16, kind="ExternalInput")  # quad-permuted rows
    cb = nc.dram_tensor("cb", [8, 8, 128, 512], BF16, kind="ExternalInput")
    bq = nc.dram_tensor("bq", [128, FT_Q], F32, kind="ExternalInput")
    bp = nc.dram_tensor("bp", [128, C], F32, kind="ExternalInput")
    ident = nc.dram_tensor("ident", [128, 128], BF16, kind="ExternalInput")
    out = nc.dram_tensor("out", [rows, C], F32, kind="ExternalOutput")

    with TileContext(nc) as tc:
        with tc.tile_pool(name="const", bufs=1) as constp, \
             tc.tile_pool(name="wpool", bufs=1) as wpool, \
             tc.tile_pool(name="stream", bufs=3) as stream, \
             tc.tile_pool(name="acts", bufs=2) as acts, \
             tc.tile_pool(name="small", bufs=3) as small, \
             tc.tile_pool(name="pbig", bufs=2, space="PSUM") as pbig, \
             tc.tile_pool(name="pattn", bufs=2, space="PSUM") as pattn, \
             tc.tile_pool(name="ppv", bufs=2, space="PSUM") as ppv, \
             tc.tile_pool(name="pden", bufs=2, space="PSUM") as pden:

            # ---- resident constants / weights
            w1_sb = wpool.tile([128, KT_X, 3 * C], BF16)
            nc.sync.dma_start(out=w1_sb, in_=w1.rearrange("(a p) f -> p a f", p=128))
            w2_sb = wpool.tile([128, KT_Y, 3 * C], BF16)
            nc.sync.dma_start(out=w2_sb, in_=w2.rearrange("(a p) f -> p a f", p=128))
            wp_sb = wpool.tile([128, 4, C], BF16)
            nc.sync.dma_start(out=wp_sb, in_=wp.rearrange("a p f -> p a f"))
            bq_sb = constp.tile([128, FT_Q], F32)
            nc.sync.dma_start(out=bq_sb, in_=bq[:, :])
            bp_sb = constp.tile([128, C], F32)
            nc.sync.dma_start(out=bp_sb, in_=bp[:, :])
            # den-matmul stationary: cols 0-31 sum partitions 0-63 (hh=0),
            # cols 32-63 sum partitions 64-127 (hh=1) -- denominator comes
            # out of the PE already replicated across the 32 d-partitions
            ones2_sb = constp.tile([128, 64], BF16)
            nc.vector.memset(ones2_sb[:, :], 0.0)
            nc.vector.memset(ones2_sb[0:64, 0:32], 1.0)
            nc.vector.memset(ones2_sb[64:128, 32:64], 1.0)
            id_sb = constp.tile([128, 128], BF16)
            nc.sync.dma_start(out=id_sb, in_=ident[:, :])

            xt_r = xt.rearrange("(a p) r -> p a r", p=128)
            yt_r = yt.rearrange("(a p) r -> p a r", p=128)

            st = {}  # per-chunk live tiles

            def emit_dma(c):
                r0 = c * ROWS_PER_CHUNK
                s = {}
                s["xt"] = stream.tile([128, KT_X, ROWS_PER_CHUNK], BF16, tag="xt", name="xt")
                nc.sync.dma_start(out=s["xt"], in_=xt_r[:, :, r0:r0 + ROWS_PER_CHUNK])
                s["yt"] = stream.tile([128, KT_Y, ROWS_PER_CHUNK], BF16, tag="yt", name="yt")
                nc.sync.dma_start(out=s["yt"], in_=yt_r[:, :, r0:r0 + ROWS_PER_CHUNK])
                s["cb"] = stream.tile([128, 8, 512], BF16, tag="cb", name="cbt")
                nc.sync.dma_start(out=s["cb"],
                                  in_=cb[c % 8].rearrange("hp p f -> p hp f"))
                s["q"] = acts.tile([128, FT_Q, ROWS_PER_CHUNK], BF16, tag="q", name="qsb")
                s["k"] = acts.tile([128, FT_Q, ROWS_PER_CHUNK], BF16, tag="k", name="ksb")
                # v lives on both partition halves (dup'd by DMA) so PV's
                # stationary base matches the moving exp slice's base 64*hh
                s["v"] = acts.tile([128, WIN_PER_CHUNK, C], BF16, tag="v", name="vsb")
                s["expT"] = acts.tile([128, 8, 512], BF16, tag="expT", name="expT")
                s["ot"] = acts.tile([128, 4 * ROWS_PER_CHUNK], BF16, tag="ot", name="otsb")
                st[c] = s

            def emit_qkv_group(c, g):
                s = st[c]
                if g < FT_Q:                      # q projection, feature tile g
                    ft = g
                    bank = pbig.tile([128, ROWS_PER_CHUNK], F32, tag="pq")
                    for kt in range(KT_X):
                        nc.tensor.matmul(
                            bank[:, :],
                            w1_sb[:, kt, 128 * ft:128 * (ft + 1)],
                            s["xt"][:, kt, :],
                            start=(kt == 0), stop=(kt == KT_X - 1))
                    nc.scalar.activation(
                        s["q"][:, ft, :], bank[:, :],
                        mybir.ActivationFunctionType.Identity,
                        bias=bq_sb[:, ft:ft + 1])
                elif g < 2 * FT_Q:                # k projection, feature tile g-4
                    ft = g - FT_Q
                    bank = pbig.tile([128, ROWS_PER_CHUNK], F32, tag="pq")
                    for kt in range(KT_X):
                        nc.tensor.matmul(
                            bank[:, :],
                            w1_sb[:, kt, C + 128 * ft:C + 128 * (ft + 1)],
                            s["xt"][:, kt, :],
                            start=(kt == 0), stop=False)
                    for kt in range(KT_Y):
                        nc.tensor.matmul(
                            bank[:, :],
                            w2_sb[:, kt, C + 128 * ft:C + 128 * (ft + 1)],
                            s["yt"][:, kt, :],
                            start=False, stop=(kt == KT_Y - 1))
                    nc.scalar.copy(s["k"][:, ft, :], bank[:, :])
                else:                             # v projection, row tile g-8
                    rt = g - 2 * FT_Q
                    bank = pbig.tile([128, C], F32, tag="pq")
                    for kt in range(KT_X):
                        nc.tensor.matmul(
                            bank[:, :],
                            s["xt"][:, kt, 128 * rt:128 * (rt + 1)],
                            w1_sb[:, kt, 2 * C:3 * C],
                            start=(kt == 0), stop=False)
                    for kt in range(KT_Y):
                        nc.tensor.matmul(
                            bank[:, :],
                            s["yt"][:, kt, 128 * rt:128 * (rt + 1)],
                            w2_sb[:, kt, 2 * C:3 * C],
                            start=False, stop=(kt == KT_Y - 1))
                    # v staged into BOTH partition halves (PV stationary for
                    # head-half hh reads partitions 64*hh..64*hh+64)
                    nc.scalar.copy(s["v"][0:64, 2 * rt, :], bank[0:64, :])
                    nc.scalar.copy(s["v"][64:128, 2 * rt, :], bank[0:64, :])
                    nc.vector.tensor_copy(s["v"][0:64, 2 * rt + 1, :], bank[64:128, :])
                    nc.vector.tensor_copy(s["v"][64:128, 2 * rt + 1, :], bank[64:128, :])

            def emit_attn_a(c, hp):
                # QK + bias + exp: the only PE->other-engine handoff; the
                # consuming den/PV matmuls are emitted several slots later
                # (emit_attn_b) so the PE FIFO never stalls on exp.
                s = st[c]
                bank = pattn.tile([128, 512], F32, tag="pattn")
                # combined rpb+mask bias (pre-divided by SCALE, incl -C0
                # shift) written first via identity matmul; the QK matmuls
                # then accumulate onto it (PE-only, no DVE in the chain)
                nc.tensor.matmul(
                    bank[:, :], id_sb[:, :], s["cb"][:, hp, :],
                    start=True, stop=False, skip_group_check=True)
                # S^T[m, n] per (window, head): stationary = k, moving = q
                for sw in range(WIN_PER_CHUNK):
                    for hh in range(2):
                        h = 2 * hp + hh
                        pq = 32 * (h % 4)
                        ft = h // 4
                        nc.tensor.matmul(
                            bank[64 * hh:64 * (hh + 1), 64 * sw:64 * (sw + 1)],
                            s["k"][pq:pq + 32, ft, 64 * sw:64 * (sw + 1)],
                            s["q"][pq:pq + 32, ft, 64 * sw:64 * (sw + 1)],
                            start=False,
                            stop=(sw == WIN_PER_CHUNK - 1 and hh == 1),
                            skip_group_check=True,
                            tile_position=(pq, 64 * hh))
                expT = s["expT"]
                nc.scalar.activation(
                    expT[:, hp, :], bank[:, :],
                    mybir.ActivationFunctionType.Exp, scale=SCALE)

            def emit_attn_b(c, hp):
                s = st[c]
                expT = s["expT"]
                pot = ppv.tile([64, 512], F32, tag="pot")
                # den bank (base partition 0 -- reciprocal_approx_fast and
                # other custom ops misread partition-offset inputs on HW):
                # den[hh] replicated over the 32 d-partitions
                dbank = pden.tile([64, 512], F32, tag="pden")
                nc.tensor.matmul(
                    dbank[:, :], ones2_sb[:, :], expT[:, hp, :],
                    start=True, stop=True, skip_group_check=True)
                # PV: U[d, n] = sum_m v[m, d] expT[m, n] (unnormalized)
                for sw in range(WIN_PER_CHUNK):
                    for hh in range(2):
                        h = 2 * hp + hh
                        nc.tensor.matmul(
                            pot[32 * hh:32 * (hh + 1), 64 * sw:64 * (sw + 1)],
                            s["v"][64 * hh:64 * (hh + 1), sw, HD * h:HD * (h + 1)],
                            expT[64 * hh:64 * (hh + 1), hp, 64 * sw:64 * (sw + 1)],
                            start=True, stop=True, skip_group_check=True,
                            tile_position=(64 * hh, 32 * hh))
                rrep = small.tile([64, 512], F32, tag="rrep")
                nc.vector.reciprocal_approx_fast(rrep[:, :], dbank[:, :])
                # stage to SBUF with fused normalization:
                # ot[p=32*(h%4)+d, (t, q=h//4, w, n)] = U * (1/den)
                dst = s["ot"][64 * (hp % 2):64 * (hp % 2) + 64, :] \
                    .rearrange("p (t q w m) -> p t q w m", t=4, q=4, w=2) \
                    [:, :, hp // 2, :, :]
                nc.vector.tensor_tensor(
                    out=dst,
                    in0=pot[0:64, :].rearrange("p (t w m) -> p t w m", t=4, w=2),
                    in1=rrep.rearrange("p (t w m) -> p t w m", t=4, w=2),
                    op=mybir.AluOpType.mult)

            def emit_proj_group(c, rt):
                s = st[c]
                r0 = c * ROWS_PER_CHUNK
                bank = pbig.tile([128, C], F32, tag="pq")
                for quad in range(4):
                    nc.tensor.matmul(
                        bank[:, :],
                        s["ot"].rearrange("p (t q f) -> p t q f", t=4, q=4)
                        [:, rt, quad, :],
                        wp_sb[:, quad, :],
                        start=(quad == 0), stop=(quad == 3))
                out_f32 = small.tile([128, C], F32, tag="outf")
                nc.vector.tensor_tensor(out=out_f32[:, :], in0=bank[:, :],
                                        in1=bp_sb[:, :], op=mybir.AluOpType.add)
                nc.sync.dma_start(
                    out=out[r0 + 128 * rt:r0 + 128 * (rt + 1), :],
                    in_=out_f32[:, :])

            # software pipeline: big qkv groups of chunk c interleaved with
            # small attention/proj groups of chunk c-1.  attn part B (den/PV,
            # needs exp of part A) trails part A by two slots so the PE FIFO
            # never waits on the ScalarE exp.
            emit_dma(0)
            for c in range(n_chunks + 1):
                # prefetch next chunk's inputs a full chunk ahead so the
                # chunk-boundary qkv matmuls never wait on DMA
                if c + 1 < n_chunks:
                    emit_dma(c + 1)
                big = [("qkv", c, g) for g in range(12)] if c < n_chunks else []
                if c > 0:
                    smalls = [("atta", c - 1, 0), ("atta", c - 1, 1)]
                    for hp in range(2, 8):
                        smalls += [("attb", c - 1, hp - 2), ("atta", c - 1, hp)]
                    smalls += [("attb", c - 1, 6), ("attb", c - 1, 7)]
                    smalls += [("proj", c - 1, rt) for rt in range(4)]
                else:
                    smalls = []
                order = []
                for i in range(max(len(big), len(smalls))):
                    if i < len(big):
                        order.append(big[i])
                    if i < len(smalls):
                        order.append(smalls[i])
                for kind, cc, idx in order:
                    if kind == "qkv":
                        emit_qkv_group(cc, idx)
                    elif kind == "atta":
                        emit_attn_a(cc, idx)
                    elif kind == "attb":
                        emit_attn_b(cc, idx)
                    else:
                        emit_proj_group(cc, idx)
                if c > 0:
                    del st[c - 1]
    nc.compile()
    return nc


_NC_CACHE = {}


def _get_nc(n_chunks):
    if n_chunks not in _NC_CACHE:
        _NC_CACHE[n_chunks] = build_nc(n_chunks)
    return _NC_CACHE[n_chunks]


def prep_shared(w_qkv1, b_qkv1, w_qkv2, b_qkv2, bias_table, rel_index, w_proj,
                b_proj, mask):
    """Host-side prep of weights/bias tables shared by all cores."""
    w1 = w_qkv1.astype(BF)
    w2 = np.zeros((CYP, 3 * C), np.float32)
    w2[:CY] = w_qkv2
    # k/v biases ride an all-ones row in the padded region of yT
    w2[CY, C:2 * C] = b_qkv1[C:2 * C] + b_qkv2[C:2 * C]
    w2[CY, 2 * C:] = b_qkv1[2 * C:] + b_qkv2[2 * C:]
    w2 = w2.astype(BF)
    # quad-permuted rows: wp[Q, p, :] = w_proj[32*(4Q + p//32) + p%32, :]
    wp = np.empty((4, 128, C), np.float32)
    for q in range(4):
        for g in range(4):
            wp[q, 32 * g:32 * (g + 1), :] = \
                w_proj[32 * (4 * q + g):32 * (4 * q + g) + 32, :]
    wp = wp.astype(BF)

    bq = b_qkv1[0:C].reshape(FT_Q, 128).T.astype(np.float32).copy()
    bp = np.broadcast_to(b_proj.astype(np.float32), (128, C)).copy()

    rpb = bias_table[rel_index.reshape(-1)].reshape(N, N, H).transpose(2, 0, 1)
    cbt = (rpb[None] + mask[:, None] - C0) / SCALE      # [w, h, n, m]
    cb6 = cbt.reshape(8, 8, 8, 2, N, N)                 # [c8, sw, hp, hh, n, m]
    # transposed bank layout: [c8, hp, (hh, m), (sw, n)]
    cbd = np.ascontiguousarray(cb6.transpose(0, 2, 3, 5, 1, 4)) \
        .reshape(8, 8, 128, 512).astype(BF)

    ident = np.eye(128, dtype=BF)
    return dict(w1=w1, w2=w2, wp=wp, bq=bq, bp=bp, cb=cbd, ident=ident)


def prep_core_inputs(x, y, shared, n_cores=N_CORES):
    """Split x, y along batch, transpose to feature-major, bf16."""
    B_, n, _ = x.shape
    rows = (B_ // n_cores) * n
    in_maps = []
    for i in range(n_cores):
        lo = i * (B_ // n_cores)
        hi = lo + B_ // n_cores
        xs = x[lo:hi].reshape(rows, CX)
        ys = y[lo:hi].reshape(rows, CY)
        xtb = np.ascontiguousarray(xs.T).astype(BF)
        ytb = np.zeros((CYP, rows), BF)
        ytb[:CY] = np.ascontiguousarray(ys.T).astype(BF)
        ytb[CY] = 1.0
        in_maps.append(dict(xt=xtb, yt=ytb, **shared))
    return in_maps


def kernel(x, y, mask, w_qkv1, b_qkv1, w_qkv2, b_qkv2, bias_table, rel_index,
           w_proj, b_proj, _n_cores=N_CORES, _trace=False):
    B_, n, _ = x.shape
    n_chunks = (B_ // _n_cores) // WIN_PER_CHUNK
    shared = prep_shared(np.asarray(w_qkv1), np.asarray(b_qkv1),
                         np.asarray(w_qkv2), np.asarray(b_qkv2),
                         np.asarray(bias_table), np.asarray(rel_index),
                         np.asarray(w_proj), np.asarray(b_proj),
                         np.asarray(mask))
    in_maps = prep_core_inputs(np.asarray(x), np.asarray(y), shared, _n_cores)
    nc = _get_nc(n_chunks)
    res = run_bass_kernel_spmd(nc, in_maps, core_ids=list(range(_n_cores)),
                               trace=_trace)
    outs = [res.results[i]["out"].reshape(B_ // _n_cores, n, C)
            for i in range(_n_cores)]
    full = np.concatenate(outs, axis=0)
    kernel.last_results = res
    return full


# revision 47
# speedup vs baseline: 1.7541x; 1.1733x over previous
"""Cross-WindowAttention Trainium2 kernel.

Full inputs -> shard batch dim over 8 NeuronCores -> bass/Tile kernel per core
-> gather. Host-side numpy does layout prep (transposes to feature-major,
bf16 conversion, combined rpb+mask bias table); the Bass kernel does all
matmul/softmax compute.

Per-core pipeline (shard = 256 windows of 64 tokens, 16384 rows):
 - qkv projections on PE in bf16, contraction over concat(x,y) for k/v.
   q,k produced feature-major [feat, rows]; v row-major per window [64, 512].
 - attention per (head-pair, 8-window chunk) in one [128, 512] PSUM bank,
   computed TRANSPOSED (S^T[m, n], stationary = k) so the softmax weights
   come out m-major and feed the PV matmul directly -- no PE transposes.
 - softmax normalization is deferred past PV: unnormalized U = exp @ v and
   den = sum_m exp (ones-stationary matmul into the same PSUM bank), then
   U * (1/den) is fused into the PSUM->SBUF staging multiply.  1/den is
   partition-replicated via GpSimd partition_broadcast (idle engine).
 - output projection with attention-output tiles stationary -> row-major
   result, biases via ones-row matmul, contiguous DMA out.

The chunk loop is software-pipelined by one chunk: the small attention/proj
matmul groups of chunk c-1 are emitted interleaved between the large qkv
matmul groups of chunk c, keeping the PE array duty cycle high enough that
the HAM activity monitor does not clock-gate it to half speed.
"""
import numpy as np
import ml_dtypes

import concourse.bacc as bacc
import concourse.mybir as mybir
from concourse.tile import TileContext
from concourse.bass_utils import run_bass_kernel_spmd

F32 = mybir.dt.float32
BF16 = mybir.dt.bfloat16
FP8 = mybir.dt.float8e4
BF = ml_dtypes.bfloat16
F8 = ml_dtypes.float8_e4m3
DR = mybir.MatmulPerfMode.DoubleRow

N_CORES = 8
B_FULL = 2048
N = 64                      # window size (tokens per window)
C = 512                     # channels
H = 16                      # heads
HD = 32                     # head dim
CX = 512                    # x feature dim
CY = 1000                   # y feature dim
CYP = 1024                  # y feature dim padded to multiple of 128
SCALE = HD ** -0.5
C0 = 3.0                    # exp shift: exp(S - C0), cancels in U/den;
                            # keeps fp8 exp outputs within e4m3 range
WS = 64.0                   # fp8 weight scale (into e4m3 normal range)

B_SHARD = B_FULL // N_CORES             # 256 windows per core
WIN_PER_CHUNK = 8
ROWS_PER_CHUNK = WIN_PER_CHUNK * N      # 512
N_CHUNKS = B_SHARD // WIN_PER_CHUNK     # 32

KT_X = CX // 128            # 4 contraction tiles from x
KT_Y = CYP // 128           # 8 contraction tiles from y (padded)
FT_Q = C // 128             # 4 feature tiles per projection output


def build_nc(n_chunks=N_CHUNKS):
    rows = n_chunks * ROWS_PER_CHUNK
    nc = bacc.Bacc("TRN2", target_bir_lowering=False)

    xt = nc.dram_tensor("xt", [CX, rows], FP8, kind="ExternalInput")
    yt = nc.dram_tensor("yt", [CYP, rows], FP8, kind="ExternalInput")
    xtb = nc.dram_tensor("xtb", [CX, rows], BF16, kind="ExternalInput")
    ytb = nc.dram_tensor("ytb", [CYP, rows], BF16, kind="ExternalInput")
    w1 = nc.dram_tensor("w1", [CX, 2 * C], FP8, kind="ExternalInput")
    w2 = nc.dram_tensor("w2", [CYP, C], FP8, kind="ExternalInput")
    wv1 = nc.dram_tensor("wv1", [CX, C], BF16, kind="ExternalInput")
    wv2 = nc.dram_tensor("wv2", [CYP, C], BF16, kind="ExternalInput")
    wp = nc.dram_tensor("wp", [4, 128, C], BF16, kind="ExternalInput")  # quad-permuted rows
    cb = nc.dram_tensor("cb", [8, 8, 128, 512], BF16, kind="ExternalInput")
    bq = nc.dram_tensor("bq", [128, FT_Q], F32, kind="ExternalInput")
    bp = nc.dram_tensor("bp", [128, C], F32, kind="ExternalInput")
    ident = nc.dram_tensor("ident", [128, 128], BF16, kind="ExternalInput")
    out = nc.dram_tensor("out", [rows, C], F32, kind="ExternalOutput")

    with TileContext(nc) as tc:
        with tc.tile_pool(name="const", bufs=1) as constp, \
             tc.tile_pool(name="wpool", bufs=1) as wpool, \
             tc.tile_pool(name="stream", bufs=3) as stream, \
             tc.tile_pool(name="acts", bufs=2) as acts, \
             tc.tile_pool(name="small", bufs=3) as small, \
             tc.tile_pool(name="pbig", bufs=2, space="PSUM") as pbig, \
             tc.tile_pool(name="pattn", bufs=2, space="PSUM") as pattn, \
             tc.tile_pool(name="ppv", bufs=2, space="PSUM") as ppv, \
             tc.tile_pool(name="pden", bufs=2, space="PSUM") as pden:

            # ---- resident constants / weights
            w1_sb = wpool.tile([128, KT_X, 2 * C], FP8)
            nc.sync.dma_start(out=w1_sb, in_=w1.rearrange("(a p) f -> p a f", p=128))
            w2_sb = wpool.tile([128, KT_Y, C], FP8)
            nc.sync.dma_start(out=w2_sb, in_=w2.rearrange("(a p) f -> p a f", p=128))
            wv1_sb = wpool.tile([128, KT_X, C], BF16)
            nc.sync.dma_start(out=wv1_sb, in_=wv1.rearrange("(a p) f -> p a f", p=128))
            wv2_sb = wpool.tile([128, KT_Y, C], BF16)
            nc.sync.dma_start(out=wv2_sb, in_=wv2.rearrange("(a p) f -> p a f", p=128))
            wp_sb = wpool.tile([128, 4, C], BF16)
            nc.sync.dma_start(out=wp_sb, in_=wp.rearrange("a p f -> p a f"))
            bq_sb = constp.tile([128, FT_Q], F32)
            nc.sync.dma_start(out=bq_sb, in_=bq[:, :])
            bp_sb = constp.tile([128, C], F32)
            nc.sync.dma_start(out=bp_sb, in_=bp[:, :])
            # den-matmul stationary: cols 0-31 sum partitions 0-63 (hh=0),
            # cols 32-63 sum partitions 64-127 (hh=1) -- denominator comes
            # out of the PE already replicated across the 32 d-partitions
            ones2_sb = constp.tile([128, 64], BF16)
            nc.vector.memset(ones2_sb[:, :], 0.0)
            nc.vector.memset(ones2_sb[0:64, 0:32], 1.0)
            nc.vector.memset(ones2_sb[64:128, 32:64], 1.0)
            id_sb = constp.tile([128, 128], BF16)
            nc.sync.dma_start(out=id_sb, in_=ident[:, :])

            xt_r = xt.rearrange("(a p) r -> p a r", p=128)
            yt_r = yt.rearrange("(a p) r -> p a r", p=128)
            xtb_r = xtb.rearrange("(a p) r -> p a r", p=128)
            ytb_r = ytb.rearrange("(a p) r -> p a r", p=128)

            st = {}  # per-chunk live tiles

            def emit_dma(c):
                r0 = c * ROWS_PER_CHUNK
                s = {}
                s["xt"] = stream.tile([128, KT_X, ROWS_PER_CHUNK], FP8, tag="xt", name="xt")
                nc.sync.dma_start(out=s["xt"], in_=xt_r[:, :, r0:r0 + ROWS_PER_CHUNK])
                s["yt"] = stream.tile([128, KT_Y, ROWS_PER_CHUNK], FP8, tag="yt", name="yt")
                nc.sync.dma_start(out=s["yt"], in_=yt_r[:, :, r0:r0 + ROWS_PER_CHUNK])
                s["xtb"] = stream.tile([128, KT_X, ROWS_PER_CHUNK], BF16, tag="xtb", name="xtb")
                nc.sync.dma_start(out=s["xtb"], in_=xtb_r[:, :, r0:r0 + ROWS_PER_CHUNK])
                s["ytb"] = stream.tile([128, KT_Y, ROWS_PER_CHUNK], BF16, tag="ytb", name="ytb")
                nc.sync.dma_start(out=s["ytb"], in_=ytb_r[:, :, r0:r0 + ROWS_PER_CHUNK])
                s["cb"] = stream.tile([128, 8, 512], BF16, tag="cb", name="cbt")
                nc.sync.dma_start(out=s["cb"],
                                  in_=cb[c % 8].rearrange("hp p f -> p hp f"))
                s["q"] = acts.tile([128, FT_Q, ROWS_PER_CHUNK], BF16, tag="q", name="qsb")
                s["k"] = acts.tile([128, FT_Q, ROWS_PER_CHUNK], BF16, tag="k", name="ksb")
                # v lives on both partition halves (dup'd by DMA) so PV's
                # stationary base matches the moving exp slice's base 64*hh
                s["v"] = acts.tile([128, WIN_PER_CHUNK, C], BF16, tag="v", name="vsb")
                s["expT"] = acts.tile([128, 8, 512], BF16, tag="expT", name="expT")
                s["ot"] = acts.tile([128, 4 * ROWS_PER_CHUNK], BF16, tag="ot", name="otsb")
                st[c] = s

            def emit_qkv_group(c, g):
                s = st[c]
                if g < FT_Q:                      # q projection, feature tile g
                    ft = g
                    bank = pbig.tile([128, ROWS_PER_CHUNK], F32, tag="pq")
                    for kt in range(0, KT_X, 2):
                        nc.tensor.matmul(
                            bank[:, :],
                            w1_sb[:, kt:kt + 2, 128 * ft:128 * (ft + 1)],
                            s["xt"][:, kt:kt + 2, :],
                            start=(kt == 0), stop=(kt == KT_X - 2),
                            perf_mode=DR)
                    nc.scalar.activation(
                        s["q"][:, ft, :], bank[:, :],
                        mybir.ActivationFunctionType.Identity,
                        bias=bq_sb[:, ft:ft + 1], scale=1.0 / WS)
                elif g < 2 * FT_Q:                # k projection, feature tile g-4
                    ft = g - FT_Q
                    bank = pbig.tile([128, ROWS_PER_CHUNK], F32, tag="pq")
                    for kt in range(0, KT_X, 2):
                        nc.tensor.matmul(
                            bank[:, :],
                            w1_sb[:, kt:kt + 2, C + 128 * ft:C + 128 * (ft + 1)],
                            s["xt"][:, kt:kt + 2, :],
                            start=(kt == 0), stop=False, perf_mode=DR)
                    for kt in range(0, KT_Y, 2):
                        nc.tensor.matmul(
                            bank[:, :],
                            w2_sb[:, kt:kt + 2, 128 * ft:128 * (ft + 1)],
                            s["yt"][:, kt:kt + 2, :],
                            start=False, stop=(kt == KT_Y - 2), perf_mode=DR)
                    nc.scalar.mul(s["k"][:, ft, :], bank[:, :], 1.0 / WS)
                else:                             # v projection (bf16), row tile g-8
                    rt = g - 2 * FT_Q
                    bank = pbig.tile([128, C], F32, tag="pq")
                    for kt in range(KT_X):
                        nc.tensor.matmul(
                            bank[:, :],
                            s["xtb"][:, kt, 128 * rt:128 * (rt + 1)],
                            wv1_sb[:, kt, :],
                            start=(kt == 0), stop=False)
                    for kt in range(KT_Y):
                        nc.tensor.matmul(
                            bank[:, :],
                            s["ytb"][:, kt, 128 * rt:128 * (rt + 1)],
                            wv2_sb[:, kt, :],
                            start=False, stop=(kt == KT_Y - 1))
                    # v staged into BOTH partition halves (PV stationary for
                    # head-half hh reads partitions 64*hh..64*hh+64)
                    nc.scalar.copy(s["v"][0:64, 2 * rt, :], bank[0:64, :])
                    nc.scalar.copy(s["v"][64:128, 2 * rt, :], bank[0:64, :])
                    nc.vector.tensor_copy(s["v"][0:64, 2 * rt + 1, :], bank[64:128, :])
                    nc.vector.tensor_copy(s["v"][64:128, 2 * rt + 1, :], bank[64:128, :])

            def emit_attn_a(c, hp):
                # QK + bias + exp: the only PE->other-engine handoff; the
                # consuming den/PV matmuls are emitted several slots later
                # (emit_attn_b) so the PE FIFO never stalls on exp.
                s = st[c]
                bank = pattn.tile([128, 512], F32, tag="pattn")
                # combined rpb+mask bias (pre-divided by SCALE, incl -C0
                # shift) written first via identity matmul; the QK matmuls
                # then accumulate onto it (PE-only, no DVE in the chain)
                nc.tensor.matmul(
                    bank[:, :], id_sb[:, :], s["cb"][:, hp, :],
                    start=True, stop=False, skip_group_check=True)
                # S^T[m, n] per (window, head): stationary = k, moving = q
                for sw in range(WIN_PER_CHUNK):
                    for hh in range(2):
                        h = 2 * hp + hh
                        pq = 32 * (h % 4)
                        ft = h // 4
                        nc.tensor.matmul(
                            bank[64 * hh:64 * (hh + 1), 64 * sw:64 * (sw + 1)],
                            s["k"][pq:pq + 32, ft, 64 * sw:64 * (sw + 1)],
                            s["q"][pq:pq + 32, ft, 64 * sw:64 * (sw + 1)],
                            start=False,
                            stop=(sw == WIN_PER_CHUNK - 1 and hh == 1),
                            skip_group_check=True,
                            tile_position=(pq, 64 * hh))
                expT = s["expT"]
                nc.scalar.activation(
                    expT[:, hp, :], bank[:, :],
                    mybir.ActivationFunctionType.Exp, scale=SCALE)

            def emit_attn_b(c, hp):
                s = st[c]
                expT = s["expT"]
                pot = ppv.tile([64, 512], F32, tag="pot")
                # den bank (base partition 0 -- reciprocal_approx_fast and
                # other custom ops misread partition-offset inputs on HW):
                # den[hh] replicated over the 32 d-partitions
                dbank = pden.tile([64, 512], F32, tag="pden")
                nc.tensor.matmul(
                    dbank[:, :], ones2_sb[:, :], expT[:, hp, :],
                    start=True, stop=True, skip_group_check=True)
                # PV: U[d, n] = sum_m v[m, d] expT[m, n] (unnormalized)
                for sw in range(WIN_PER_CHUNK):
                    for hh in range(2):
                        h = 2 * hp + hh
                        nc.tensor.matmul(
                            pot[32 * hh:32 * (hh + 1), 64 * sw:64 * (sw + 1)],
                            s["v"][64 * hh:64 * (hh + 1), sw, HD * h:HD * (h + 1)],
                            expT[64 * hh:64 * (hh + 1), hp, 64 * sw:64 * (sw + 1)],
                            start=True, stop=True, skip_group_check=True,
                            tile_position=(64 * hh, 32 * hh))
                rrep = small.tile([64, 512], F32, tag="rrep")
                nc.vector.reciprocal_approx_fast(rrep[:, :], dbank[:, :])
                # stage to SBUF with fused normalization:
                # ot[p=32*(h%4)+d, (t, q=h//4, w, n)] = U * (1/den)
                dst = s["ot"][64 * (hp % 2):64 * (hp % 2) + 64, :] \
                    .rearrange("p (t q w m) -> p t q w m", t=4, q=4, w=2) \
                    [:, :, hp // 2, :, :]
                nc.vector.tensor_tensor(
                    out=dst,
                    in0=pot[0:64, :].rearrange("p (t w m) -> p t w m", t=4, w=2),
                    in1=rrep.rearrange("p (t w m) -> p t w m", t=4, w=2),
                    op=mybir.AluOpType.mult)

            def emit_proj_group(c, rt):
                s = st[c]
                r0 = c * ROWS_PER_CHUNK
                bank = pbig.tile([128, C], F32, tag="pq")
                for quad in range(4):
                    nc.tensor.matmul(
                        bank[:, :],
                        s["ot"].rearrange("p (t q f) -> p t q f", t=4, q=4)
                        [:, rt, quad, :],
                        wp_sb[:, quad, :],
                        start=(quad == 0), stop=(quad == 3))
                out_f32 = small.tile([128, C], F32, tag="outf")
                nc.vector.tensor_tensor(out=out_f32[:, :], in0=bank[:, :],
                                        in1=bp_sb[:, :], op=mybir.AluOpType.add)
                nc.sync.dma_start(
                    out=out[r0 + 128 * rt:r0 + 128 * (rt + 1), :],
                    in_=out_f32[:, :])

            # software pipeline: big qkv groups of chunk c interleaved with
            # small attention/proj groups of chunk c-1.  attn part B (den/PV,
            # needs exp of part A) trails part A by two slots so the PE FIFO
            # never waits on the ScalarE exp.
            emit_dma(0)
            for c in range(n_chunks + 1):
                # prefetch next chunk's inputs a full chunk ahead so the
                # chunk-boundary qkv matmuls never wait on DMA
                if c + 1 < n_chunks:
                    emit_dma(c + 1)
                big = [("qkv", c, g) for g in range(12)] if c < n_chunks else []
                if c > 0:
                    smalls = [("atta", c - 1, 0), ("atta", c - 1, 1)]
                    for hp in range(2, 8):
                        smalls += [("attb", c - 1, hp - 2), ("atta", c - 1, hp)]
                    smalls += [("attb", c - 1, 6), ("attb", c - 1, 7)]
                    smalls += [("proj", c - 1, rt) for rt in range(4)]
                else:
                    smalls = []
                order = []
                for i in range(max(len(big), len(smalls))):
                    if i < len(big):
                        order.append(big[i])
                    if i < len(smalls):
                        order.append(smalls[i])
                for kind, cc, idx in order:
                    if kind == "qkv":
                        emit_qkv_group(cc, idx)
                    elif kind == "atta":
                        emit_attn_a(cc, idx)
                    elif kind == "attb":
                        emit_attn_b(cc, idx)
                    else:
                        emit_proj_group(cc, idx)
                if c > 0:
                    del st[c - 1]
    nc.compile()
    return nc


_NC_CACHE = {}


def _get_nc(n_chunks):
    if n_chunks not in _NC_CACHE:
        _NC_CACHE[n_chunks] = build_nc(n_chunks)
    return _NC_CACHE[n_chunks]


def prep_shared(w_qkv1, b_qkv1, w_qkv2, b_qkv2, bias_table, rel_index, w_proj,
                b_proj, mask):
    """Host-side prep of weights/bias tables shared by all cores."""
    # q+k weight columns in fp8 (scaled by WS into e4m3 normal range)
    w1 = np.clip(w_qkv1[:, 0:2 * C] * WS, -240, 240).astype(F8)
    w2k = np.zeros((CYP, C), np.float32)
    w2k[:CY] = w_qkv2[:, C:2 * C]
    # k bias rides an all-ones row in the padded region of yT
    w2k[CY] = b_qkv1[C:2 * C] + b_qkv2[C:2 * C]
    w2 = np.clip(w2k * WS, -240, 240).astype(F8)
    # v weight columns stay bf16
    wv1 = w_qkv1[:, 2 * C:].astype(BF)
    wv2f = np.zeros((CYP, C), np.float32)
    wv2f[:CY] = w_qkv2[:, 2 * C:]
    wv2f[CY] = b_qkv1[2 * C:] + b_qkv2[2 * C:]
    wv2 = wv2f.astype(BF)
    # quad-permuted rows: wp[Q, p, :] = w_proj[32*(4Q + p//32) + p%32, :]
    wp = np.empty((4, 128, C), np.float32)
    for q in range(4):
        for g in range(4):
            wp[q, 32 * g:32 * (g + 1), :] = \
                w_proj[32 * (4 * q + g):32 * (4 * q + g) + 32, :]
    wp = wp.astype(BF)

    bq = b_qkv1[0:C].reshape(FT_Q, 128).T.astype(np.float32).copy()
    bp = np.broadcast_to(b_proj.astype(np.float32), (128, C)).copy()

    rpb = bias_table[rel_index.reshape(-1)].reshape(N, N, H).transpose(2, 0, 1)
    cbt = (rpb[None] + mask[:, None] - C0) / SCALE      # [w, h, n, m]
    cb6 = cbt.reshape(8, 8, 8, 2, N, N)                 # [c8, sw, hp, hh, n, m]
    # transposed bank layout: [c8, hp, (hh, m), (sw, n)]
    cbd = np.ascontiguousarray(cb6.transpose(0, 2, 3, 5, 1, 4)) \
        .reshape(8, 8, 128, 512).astype(BF)

    ident = np.eye(128, dtype=BF)
    return dict(w1=w1, w2=w2, wv1=wv1, wv2=wv2, wp=wp, bq=bq, bp=bp, cb=cbd,
                ident=ident)


def prep_core_inputs(x, y, shared, n_cores=N_CORES):
    """Split x, y along batch, transpose to feature-major, bf16."""
    B_, n, _ = x.shape
    rows = (B_ // n_cores) * n
    in_maps = []
    for i in range(n_cores):
        lo = i * (B_ // n_cores)
        hi = lo + B_ // n_cores
        xs = x[lo:hi].reshape(rows, CX)
        ys = y[lo:hi].reshape(rows, CY)
        xT = np.ascontiguousarray(xs.T)
        yT = np.ascontiguousarray(ys.T)
        xt8 = np.clip(xT, -240, 240).astype(F8)
        yt8 = np.zeros((CYP, rows), F8)
        yt8[:CY] = np.clip(yT, -240, 240).astype(F8)
        yt8[CY] = 1.0
        xtb = xT.astype(BF)
        ytb = np.zeros((CYP, rows), BF)
        ytb[:CY] = yT.astype(BF)
        ytb[CY] = 1.0
        in_maps.append(dict(xt=xt8, yt=yt8, xtb=xtb, ytb=ytb, **shared))
    return in_maps


def kernel(x, y, mask, w_qkv1, b_qkv1, w_qkv2, b_qkv2, bias_table, rel_index,
           w_proj, b_proj, _n_cores=N_CORES, _trace=False):
    B_, n, _ = x.shape
    n_chunks = (B_ // _n_cores) // WIN_PER_CHUNK
    shared = prep_shared(np.asarray(w_qkv1), np.asarray(b_qkv1),
                         np.asarray(w_qkv2), np.asarray(b_qkv2),
                         np.asarray(bias_table), np.asarray(rel_index),
                         np.asarray(w_proj), np.asarray(b_proj),
                         np.asarray(mask))
    in_maps = prep_core_inputs(np.asarray(x), np.asarray(y), shared, _n_cores)
    nc = _get_nc(n_chunks)
    res = run_bass_kernel_spmd(nc, in_maps, core_ids=list(range(_n_cores)),
                               trace=_trace)
    outs = [res.results[i]["out"].reshape(B_ // _n_cores, n, C)
            for i in range(_n_cores)]
    full = np.concatenate(outs, axis=0)
    kernel.last_results = res
    return full


# revision 50
# speedup vs baseline: 1.8621x; 1.0616x over previous
"""Cross-WindowAttention Trainium2 kernel.

Full inputs -> shard batch dim over 8 NeuronCores -> bass/Tile kernel per core
-> gather. Host-side numpy does layout prep (transposes to feature-major,
bf16 conversion, combined rpb+mask bias table); the Bass kernel does all
matmul/softmax compute.

Per-core pipeline (shard = 256 windows of 64 tokens, 16384 rows):
 - qkv projections on PE in bf16, contraction over concat(x,y) for k/v.
   q,k produced feature-major [feat, rows]; v row-major per window [64, 512].
 - attention per (head-pair, 8-window chunk) in one [128, 512] PSUM bank,
   computed TRANSPOSED (S^T[m, n], stationary = k) so the softmax weights
   come out m-major and feed the PV matmul directly -- no PE transposes.
 - softmax normalization is deferred past PV: unnormalized U = exp @ v and
   den = sum_m exp (ones-stationary matmul into the same PSUM bank), then
   U * (1/den) is fused into the PSUM->SBUF staging multiply.  1/den is
   partition-replicated via GpSimd partition_broadcast (idle engine).
 - output projection with attention-output tiles stationary -> row-major
   result, biases via ones-row matmul, contiguous DMA out.

The chunk loop is software-pipelined by one chunk: the small attention/proj
matmul groups of chunk c-1 are emitted interleaved between the large qkv
matmul groups of chunk c, keeping the PE array duty cycle high enough that
the HAM activity monitor does not clock-gate it to half speed.
"""
import numpy as np
import ml_dtypes

import concourse.bacc as bacc
import concourse.mybir as mybir
from concourse.tile import TileContext
from concourse.bass_utils import run_bass_kernel_spmd

F32 = mybir.dt.float32
BF16 = mybir.dt.bfloat16
FP8 = mybir.dt.float8e4
BF = ml_dtypes.bfloat16
F8 = ml_dtypes.float8_e4m3
DR = mybir.MatmulPerfMode.DoubleRow

N_CORES = 8
B_FULL = 2048
N = 64                      # window size (tokens per window)
C = 512                     # channels
H = 16                      # heads
HD = 32                     # head dim
CX = 512                    # x feature dim
CY = 1000                   # y feature dim
CYP = 1024                  # y feature dim padded to multiple of 128
SCALE = HD ** -0.5
C0 = 3.0                    # exp shift: exp(S - C0), cancels in U/den;
                            # keeps fp8 exp outputs within e4m3 range
WS = 64.0                   # fp8 weight scale (into e4m3 normal range)

B_SHARD = B_FULL // N_CORES             # 256 windows per core
WIN_PER_CHUNK = 8
ROWS_PER_CHUNK = WIN_PER_CHUNK * N      # 512
N_CHUNKS = B_SHARD // WIN_PER_CHUNK     # 32

KT_X = CX // 128            # 4 contraction tiles from x
KT_Y = CYP // 128           # 8 contraction tiles from y (padded)
FT_Q = C // 128             # 4 feature tiles per projection output


def build_nc(n_chunks=N_CHUNKS):
    rows = n_chunks * ROWS_PER_CHUNK
    nc = bacc.Bacc("TRN2", target_bir_lowering=False)

    xt = nc.dram_tensor("xt", [CX, rows], FP8, kind="ExternalInput")
    yt = nc.dram_tensor("yt", [CYP, rows], FP8, kind="ExternalInput")
    xtb = nc.dram_tensor("xtb", [CX, rows], BF16, kind="ExternalInput")
    ytb = nc.dram_tensor("ytb", [CYP, rows], BF16, kind="ExternalInput")
    w1 = nc.dram_tensor("w1", [CX, 2 * C], FP8, kind="ExternalInput")
    w2 = nc.dram_tensor("w2", [CYP, C], FP8, kind="ExternalInput")
    wv1 = nc.dram_tensor("wv1", [CX, C], BF16, kind="ExternalInput")
    wv2 = nc.dram_tensor("wv2", [CYP, C], BF16, kind="ExternalInput")
    wp = nc.dram_tensor("wp", [4, 128, C], BF16, kind="ExternalInput")  # quad-permuted rows
    cb = nc.dram_tensor("cb", [8, 8, 128, 512], BF16, kind="ExternalInput")
    bq = nc.dram_tensor("bq", [128, FT_Q], F32, kind="ExternalInput")
    bp = nc.dram_tensor("bp", [128, C], F32, kind="ExternalInput")
    ident = nc.dram_tensor("ident", [128, 128], BF16, kind="ExternalInput")
    out = nc.dram_tensor("out", [rows, C], F32, kind="ExternalOutput")

    with TileContext(nc) as tc:
        with tc.tile_pool(name="const", bufs=1) as constp, \
             tc.tile_pool(name="wpool", bufs=1) as wpool, \
             tc.tile_pool(name="stream", bufs=3) as stream, \
             tc.tile_pool(name="acts", bufs=2) as acts, \
             tc.tile_pool(name="small", bufs=3) as small, \
             tc.tile_pool(name="pbig", bufs=3, space="PSUM") as pbig, \
             tc.tile_pool(name="pattn", bufs=3, space="PSUM") as pattn, \
             tc.tile_pool(name="ppv", bufs=2, space="PSUM") as ppv:

            # ---- resident constants / weights
            w1_sb = wpool.tile([128, KT_X, 2 * C], FP8)
            nc.sync.dma_start(out=w1_sb, in_=w1.rearrange("(a p) f -> p a f", p=128))
            w2_sb = wpool.tile([128, KT_Y, C], FP8)
            nc.sync.dma_start(out=w2_sb, in_=w2.rearrange("(a p) f -> p a f", p=128))
            wv1_sb = wpool.tile([128, KT_X, C], BF16)
            nc.sync.dma_start(out=wv1_sb, in_=wv1.rearrange("(a p) f -> p a f", p=128))
            wv2_sb = wpool.tile([128, KT_Y, C], BF16)
            nc.sync.dma_start(out=wv2_sb, in_=wv2.rearrange("(a p) f -> p a f", p=128))
            wp_sb = wpool.tile([128, 4, C], BF16)
            nc.sync.dma_start(out=wp_sb, in_=wp.rearrange("a p f -> p a f"))
            bq_sb = constp.tile([128, FT_Q], F32)
            nc.sync.dma_start(out=bq_sb, in_=bq[:, :])
            bp_sb = constp.tile([128, C], F32)
            nc.sync.dma_start(out=bp_sb, in_=bp[:, :])
            # den-matmul stationary: cols 0-31 sum partitions 0-63 (hh=0),
            # cols 32-63 sum partitions 64-127 (hh=1) -- denominator comes
            # out of the PE already replicated across the 32 d-partitions
            ones2_sb = constp.tile([128, 64], BF16)
            nc.vector.memset(ones2_sb[:, :], 0.0)
            nc.vector.memset(ones2_sb[0:64, 0:32], 1.0)
            nc.vector.memset(ones2_sb[64:128, 32:64], 1.0)
            id_sb = constp.tile([128, 128], BF16)
            nc.sync.dma_start(out=id_sb, in_=ident[:, :])

            xt_r = xt.rearrange("(a p) r -> p a r", p=128)
            yt_r = yt.rearrange("(a p) r -> p a r", p=128)
            xtb_r = xtb.rearrange("(a p) r -> p a r", p=128)
            ytb_r = ytb.rearrange("(a p) r -> p a r", p=128)

            st = {}  # per-chunk live tiles

            def emit_dma(c):
                r0 = c * ROWS_PER_CHUNK
                s = {}
                s["xt"] = stream.tile([128, KT_X, ROWS_PER_CHUNK], FP8, tag="xt", name="xt")
                nc.sync.dma_start(out=s["xt"], in_=xt_r[:, :, r0:r0 + ROWS_PER_CHUNK])
                s["yt"] = stream.tile([128, KT_Y, ROWS_PER_CHUNK], FP8, tag="yt", name="yt")
                nc.sync.dma_start(out=s["yt"], in_=yt_r[:, :, r0:r0 + ROWS_PER_CHUNK])
                s["xtb"] = stream.tile([128, KT_X, ROWS_PER_CHUNK], BF16, tag="xtb", name="xtb")
                nc.sync.dma_start(out=s["xtb"], in_=xtb_r[:, :, r0:r0 + ROWS_PER_CHUNK])
                s["ytb"] = stream.tile([128, KT_Y, ROWS_PER_CHUNK], BF16, tag="ytb", name="ytb")
                nc.sync.dma_start(out=s["ytb"], in_=ytb_r[:, :, r0:r0 + ROWS_PER_CHUNK])
                s["cb"] = stream.tile([128, 8, 512], BF16, tag="cb", name="cbt")
                nc.sync.dma_start(out=s["cb"],
                                  in_=cb[c % 8].rearrange("hp p f -> p hp f"))
                s["q"] = acts.tile([128, FT_Q, ROWS_PER_CHUNK], BF16, tag="q", name="qsb")
                s["k"] = acts.tile([128, FT_Q, ROWS_PER_CHUNK], BF16, tag="k", name="ksb")
                # v lives on both partition halves (dup'd by DMA) so PV's
                # stationary base matches the moving exp slice's base 64*hh
                s["v"] = acts.tile([128, WIN_PER_CHUNK, C], BF16, tag="v", name="vsb")
                s["expT"] = acts.tile([128, 8, 512], BF16, tag="expT", name="expT")
                s["ot"] = acts.tile([128, 4 * ROWS_PER_CHUNK], BF16, tag="ot", name="otsb")
                st[c] = s

            def emit_qkv_group(c, g):
                s = st[c]
                if g < FT_Q:                      # q projection, feature tile g
                    ft = g
                    bank = pbig.tile([128, ROWS_PER_CHUNK], F32, tag="pq")
                    for kt in range(0, KT_X, 2):
                        nc.tensor.matmul(
                            bank[:, :],
                            w1_sb[:, kt:kt + 2, 128 * ft:128 * (ft + 1)],
                            s["xt"][:, kt:kt + 2, :],
                            start=(kt == 0), stop=(kt == KT_X - 2),
                            perf_mode=DR)
                    nc.scalar.activation(
                        s["q"][:, ft, :], bank[:, :],
                        mybir.ActivationFunctionType.Identity,
                        bias=bq_sb[:, ft:ft + 1], scale=1.0 / WS)
                elif g < 2 * FT_Q:                # k projection, feature tile g-4
                    ft = g - FT_Q
                    bank = pbig.tile([128, ROWS_PER_CHUNK], F32, tag="pq")
                    for kt in range(0, KT_X, 2):
                        nc.tensor.matmul(
                            bank[:, :],
                            w1_sb[:, kt:kt + 2, C + 128 * ft:C + 128 * (ft + 1)],
                            s["xt"][:, kt:kt + 2, :],
                            start=(kt == 0), stop=False, perf_mode=DR)
                    for kt in range(0, KT_Y, 2):
                        nc.tensor.matmul(
                            bank[:, :],
                            w2_sb[:, kt:kt + 2, 128 * ft:128 * (ft + 1)],
                            s["yt"][:, kt:kt + 2, :],
                            start=False, stop=(kt == KT_Y - 2), perf_mode=DR)
                    nc.scalar.mul(s["k"][:, ft, :], bank[:, :], 1.0 / WS)
                else:                             # v projection (bf16), row tile g-8
                    rt = g - 2 * FT_Q
                    bank = pbig.tile([128, C], F32, tag="pq")
                    for kt in range(KT_X):
                        nc.tensor.matmul(
                            bank[:, :],
                            s["xtb"][:, kt, 128 * rt:128 * (rt + 1)],
                            wv1_sb[:, kt, :],
                            start=(kt == 0), stop=False)
                    for kt in range(KT_Y):
                        nc.tensor.matmul(
                            bank[:, :],
                            s["ytb"][:, kt, 128 * rt:128 * (rt + 1)],
                            wv2_sb[:, kt, :],
                            start=False, stop=(kt == KT_Y - 1))
                    # v staged into BOTH partition halves (PV stationary for
                    # head-half hh reads partitions 64*hh..64*hh+64)
                    nc.scalar.copy(s["v"][0:64, 2 * rt, :], bank[0:64, :])
                    nc.scalar.copy(s["v"][64:128, 2 * rt, :], bank[0:64, :])
                    nc.vector.tensor_copy(s["v"][0:64, 2 * rt + 1, :], bank[64:128, :])
                    nc.vector.tensor_copy(s["v"][64:128, 2 * rt + 1, :], bank[64:128, :])

            def emit_attn_a(c, hp):
                # QK + bias + exp: the only PE->other-engine handoff; the
                # consuming den/PV matmuls are emitted several slots later
                # (emit_attn_b) so the PE FIFO never stalls on exp.
                s = st[c]
                bank = pattn.tile([128, 512], F32, tag="pattn")
                # combined rpb+mask bias (pre-divided by SCALE, incl -C0
                # shift) written first via identity matmul; the QK matmuls
                # then accumulate onto it (PE-only, no DVE in the chain)
                nc.tensor.matmul(
                    bank[:, :], id_sb[:, :], s["cb"][:, hp, :],
                    start=True, stop=False, skip_group_check=True)
                # S^T[m, n] per (window, head): stationary = k, moving = q
                for sw in range(WIN_PER_CHUNK):
                    for hh in range(2):
                        h = 2 * hp + hh
                        pq = 32 * (h % 4)
                        ft = h // 4
                        nc.tensor.matmul(
                            bank[64 * hh:64 * (hh + 1), 64 * sw:64 * (sw + 1)],
                            s["k"][pq:pq + 32, ft, 64 * sw:64 * (sw + 1)],
                            s["q"][pq:pq + 32, ft, 64 * sw:64 * (sw + 1)],
                            start=False,
                            stop=(sw == WIN_PER_CHUNK - 1 and hh == 1),
                            skip_group_check=True,
                            tile_position=(pq, 64 * hh))
                expT = s["expT"]
                nc.scalar.activation(
                    expT[:, hp, :], bank[:, :],
                    mybir.ActivationFunctionType.Exp, scale=SCALE)

            def emit_attn_b(c, hp):
                s = st[c]
                expT = s["expT"]
                # one bank: den at partitions 0-63 (base 0 --
                # reciprocal_approx_fast misreads partition-offset inputs on
                # HW), unnormalized U at partitions 64-127
                pot = ppv.tile([128, 512], F32, tag="pot")
                nc.tensor.matmul(
                    pot[0:64, :], ones2_sb[:, :], expT[:, hp, :],
                    start=True, stop=True, skip_group_check=True)
                # PV: U[d, n] = sum_m v[m, d] expT[m, n] (unnormalized)
                for sw in range(WIN_PER_CHUNK):
                    for hh in range(2):
                        h = 2 * hp + hh
                        nc.tensor.matmul(
                            pot[64 + 32 * hh:96 + 32 * hh, 64 * sw:64 * (sw + 1)],
                            s["v"][64 * hh:64 * (hh + 1), sw, HD * h:HD * (h + 1)],
                            expT[64 * hh:64 * (hh + 1), hp, 64 * sw:64 * (sw + 1)],
                            start=True, stop=True, skip_group_check=True,
                            tile_position=(64 * hh, 64 + 32 * hh))
                rrep = small.tile([64, 512], F32, tag="rrep")
                nc.vector.reciprocal_approx_fast(rrep[:, :], pot[0:64, :])
                # stage to SBUF with fused normalization:
                # ot[p=32*(h%4)+d, (t, q=h//4, w, n)] = U * (1/den)
                dst = s["ot"][64 * (hp % 2):64 * (hp % 2) + 64, :] \
                    .rearrange("p (t q w m) -> p t q w m", t=4, q=4, w=2) \
                    [:, :, hp // 2, :, :]
                nc.vector.tensor_tensor(
                    out=dst,
                    in0=pot[64:128, :].rearrange("p (t w m) -> p t w m", t=4, w=2),
                    in1=rrep.rearrange("p (t w m) -> p t w m", t=4, w=2),
                    op=mybir.AluOpType.mult)

            def emit_proj_group(c, rt):
                s = st[c]
                r0 = c * ROWS_PER_CHUNK
                bank = pbig.tile([128, C], F32, tag="pq")
                for quad in range(4):
                    nc.tensor.matmul(
                        bank[:, :],
                        s["ot"].rearrange("p (t q f) -> p t q f", t=4, q=4)
                        [:, rt, quad, :],
                        wp_sb[:, quad, :],
                        start=(quad == 0), stop=(quad == 3))
                out_f32 = small.tile([128, C], F32, tag="outf")
                nc.vector.tensor_tensor(out=out_f32[:, :], in0=bank[:, :],
                                        in1=bp_sb[:, :], op=mybir.AluOpType.add)
                nc.sync.dma_start(
                    out=out[r0 + 128 * rt:r0 + 128 * (rt + 1), :],
                    in_=out_f32[:, :])

            # software pipeline: big qkv groups of chunk c interleaved with
            # small attention/proj groups of chunk c-1.  attn part B (den/PV,
            # needs exp of part A) trails part A by two slots so the PE FIFO
            # never waits on the ScalarE exp.
            emit_dma(0)
            for c in range(n_chunks + 1):
                # prefetch next chunk's inputs a full chunk ahead so the
                # chunk-boundary qkv matmuls never wait on DMA
                if c + 1 < n_chunks:
                    emit_dma(c + 1)
                big = [("qkv", c, g) for g in range(12)] if c < n_chunks else []
                if c > 0:
                    smalls = [("atta", c - 1, 0), ("atta", c - 1, 1)]
                    for hp in range(2, 8):
                        smalls += [("attb", c - 1, hp - 2), ("atta", c - 1, hp)]
                    smalls += [("attb", c - 1, 6), ("attb", c - 1, 7)]
                    smalls += [("proj", c - 1, rt) for rt in range(4)]
                else:
                    smalls = []
                order = []
                for i in range(max(len(big), len(smalls))):
                    if i < len(big):
                        order.append(big[i])
                    if i < len(smalls):
                        order.append(smalls[i])
                for kind, cc, idx in order:
                    if kind == "qkv":
                        emit_qkv_group(cc, idx)
                    elif kind == "atta":
                        emit_attn_a(cc, idx)
                    elif kind == "attb":
                        emit_attn_b(cc, idx)
                    else:
                        emit_proj_group(cc, idx)
                if c > 0:
                    del st[c - 1]
    nc.compile()
    return nc


_NC_CACHE = {}


def _get_nc(n_chunks):
    if n_chunks not in _NC_CACHE:
        _NC_CACHE[n_chunks] = build_nc(n_chunks)
    return _NC_CACHE[n_chunks]


def prep_shared(w_qkv1, b_qkv1, w_qkv2, b_qkv2, bias_table, rel_index, w_proj,
                b_proj, mask):
    """Host-side prep of weights/bias tables shared by all cores."""
    # q+k weight columns in fp8 (scaled by WS into e4m3 normal range)
    w1 = np.clip(w_qkv1[:, 0:2 * C] * WS, -240, 240).astype(F8)
    w2k = np.zeros((CYP, C), np.float32)
    w2k[:CY] = w_qkv2[:, C:2 * C]
    # k bias rides an all-ones row in the padded region of yT
    w2k[CY] = b_qkv1[C:2 * C] + b_qkv2[C:2 * C]
    w2 = np.clip(w2k * WS, -240, 240).astype(F8)
    # v weight columns stay bf16
    wv1 = w_qkv1[:, 2 * C:].astype(BF)
    wv2f = np.zeros((CYP, C), np.float32)
    wv2f[:CY] = w_qkv2[:, 2 * C:]
    wv2f[CY] = b_qkv1[2 * C:] + b_qkv2[2 * C:]
    wv2 = wv2f.astype(BF)
    # quad-permuted rows: wp[Q, p, :] = w_proj[32*(4Q + p//32) + p%32, :]
    wp = np.empty((4, 128, C), np.float32)
    for q in range(4):
        for g in range(4):
            wp[q, 32 * g:32 * (g + 1), :] = \
                w_proj[32 * (4 * q + g):32 * (4 * q + g) + 32, :]
    wp = wp.astype(BF)

    bq = b_qkv1[0:C].reshape(FT_Q, 128).T.astype(np.float32).copy()
    bp = np.broadcast_to(b_proj.astype(np.float32), (128, C)).copy()

    rpb = bias_table[rel_index.reshape(-1)].reshape(N, N, H).transpose(2, 0, 1)
    cbt = (rpb[None] + mask[:, None] - C0) / SCALE      # [w, h, n, m]
    cb6 = cbt.reshape(8, 8, 8, 2, N, N)                 # [c8, sw, hp, hh, n, m]
    # transposed bank layout: [c8, hp, (hh, m), (sw, n)]
    cbd = np.ascontiguousarray(cb6.transpose(0, 2, 3, 5, 1, 4)) \
        .reshape(8, 8, 128, 512).astype(BF)

    ident = np.eye(128, dtype=BF)
    return dict(w1=w1, w2=w2, wv1=wv1, wv2=wv2, wp=wp, bq=bq, bp=bp, cb=cbd,
                ident=ident)


def prep_core_inputs(x, y, shared, n_cores=N_CORES):
    """Split x, y along batch, transpose to feature-major, bf16."""
    B_, n, _ = x.shape
    rows = (B_ // n_cores) * n
    in_maps = []
    for i in range(n_cores):
        lo = i * (B_ // n_cores)
        hi = lo + B_ // n_cores
        xs = x[lo:hi].reshape(rows, CX)
        ys = y[lo:hi].reshape(rows, CY)
        xT = np.ascontiguousarray(xs.T)
        yT = np.ascontiguousarray(ys.T)
        xt8 = np.clip(xT, -240, 240).astype(F8)
        yt8 = np.zeros((CYP, rows), F8)
        yt8[:CY] = np.clip(yT, -240, 240).astype(F8)
        yt8[CY] = 1.0
        xtb = xT.astype(BF)
        ytb = np.zeros((CYP, rows), BF)
        ytb[:CY] = yT.astype(BF)
        ytb[CY] = 1.0
        in_maps.append(dict(xt=xt8, yt=yt8, xtb=xtb, ytb=ytb, **shared))
    return in_maps


def kernel(x, y, mask, w_qkv1, b_qkv1, w_qkv2, b_qkv2, bias_table, rel_index,
           w_proj, b_proj, _n_cores=N_CORES, _trace=False):
    B_, n, _ = x.shape
    n_chunks = (B_ // _n_cores) // WIN_PER_CHUNK
    shared = prep_shared(np.asarray(w_qkv1), np.asarray(b_qkv1),
                         np.asarray(w_qkv2), np.asarray(b_qkv2),
                         np.asarray(bias_table), np.asarray(rel_index),
                         np.asarray(w_proj), np.asarray(b_proj),
                         np.asarray(mask))
    in_maps = prep_core_inputs(np.asarray(x), np.asarray(y), shared, _n_cores)
    nc = _get_nc(n_chunks)
    res = run_bass_kernel_spmd(nc, in_maps, core_ids=list(range(_n_cores)),
                               trace=_trace)
    outs = [res.results[i]["out"].reshape(B_ // _n_cores, n, C)
            for i in range(_n_cores)]
    full = np.concatenate(outs, axis=0)
    kernel.last_results = res
    return full


# revision 54
# speedup vs baseline: 1.8790x; 1.0091x over previous
"""Cross-WindowAttention Trainium2 kernel.

Full inputs -> shard batch dim over 8 NeuronCores -> bass/Tile kernel per core
-> gather. Host-side numpy does layout prep (transposes to feature-major,
bf16 conversion, combined rpb+mask bias table); the Bass kernel does all
matmul/softmax compute.

Per-core pipeline (shard = 256 windows of 64 tokens, 16384 rows):
 - qkv projections on PE in bf16, contraction over concat(x,y) for k/v.
   q,k produced feature-major [feat, rows]; v row-major per window [64, 512].
 - attention per (head-pair, 8-window chunk) in one [128, 512] PSUM bank,
   computed TRANSPOSED (S^T[m, n], stationary = k) so the softmax weights
   come out m-major and feed the PV matmul directly -- no PE transposes.
 - softmax normalization is deferred past PV: unnormalized U = exp @ v and
   den = sum_m exp (ones-stationary matmul into the same PSUM bank), then
   U * (1/den) is fused into the PSUM->SBUF staging multiply.  1/den is
   partition-replicated via GpSimd partition_broadcast (idle engine).
 - output projection with attention-output tiles stationary -> row-major
   result, biases via ones-row matmul, contiguous DMA out.

The chunk loop is software-pipelined by one chunk: the small attention/proj
matmul groups of chunk c-1 are emitted interleaved between the large qkv
matmul groups of chunk c, keeping the PE array duty cycle high enough that
the HAM activity monitor does not clock-gate it to half speed.
"""
import numpy as np
import ml_dtypes

import concourse.bacc as bacc
import concourse.mybir as mybir
from concourse.tile import TileContext
from concourse.bass_utils import run_bass_kernel_spmd

F32 = mybir.dt.float32
BF16 = mybir.dt.bfloat16
FP8 = mybir.dt.float8e4
BF = ml_dtypes.bfloat16
F8 = ml_dtypes.float8_e4m3
DR = mybir.MatmulPerfMode.DoubleRow

N_CORES = 8
B_FULL = 2048
N = 64                      # window size (tokens per window)
C = 512                     # channels
H = 16                      # heads
HD = 32                     # head dim
CX = 512                    # x feature dim
CY = 1000                   # y feature dim
CYP = 1024                  # y feature dim padded to multiple of 128
SCALE = HD ** -0.5
C0 = 3.0                    # exp shift: exp(S - C0), cancels in U/den;
                            # keeps fp8 exp outputs within e4m3 range
WS = 64.0                   # fp8 weight scale (into e4m3 normal range)

B_SHARD = B_FULL // N_CORES             # 256 windows per core
WIN_PER_CHUNK = 8
ROWS_PER_CHUNK = WIN_PER_CHUNK * N      # 512
N_CHUNKS = B_SHARD // WIN_PER_CHUNK     # 32

KT_X = CX // 128            # 4 contraction tiles from x
KT_Y = CYP // 128           # 8 contraction tiles from y (padded)
FT_Q = C // 128             # 4 feature tiles per projection output


def build_nc(n_chunks=N_CHUNKS):
    rows = n_chunks * ROWS_PER_CHUNK
    nc = bacc.Bacc("TRN2", target_bir_lowering=False)

    xt = nc.dram_tensor("xt", [CX, rows], FP8, kind="ExternalInput")
    yt = nc.dram_tensor("yt", [CYP, rows], FP8, kind="ExternalInput")
    xtb = nc.dram_tensor("xtb", [CX, rows], BF16, kind="ExternalInput")
    ytb = nc.dram_tensor("ytb", [CYP, rows], BF16, kind="ExternalInput")
    w1 = nc.dram_tensor("w1", [CX, 2 * C], FP8, kind="ExternalInput")
    w2 = nc.dram_tensor("w2", [CYP, C], FP8, kind="ExternalInput")
    wv1 = nc.dram_tensor("wv1", [CX, C], BF16, kind="ExternalInput")
    wv2 = nc.dram_tensor("wv2", [CYP, C], BF16, kind="ExternalInput")
    wp = nc.dram_tensor("wp", [4, 128, C], BF16, kind="ExternalInput")  # quad-permuted rows
    cb = nc.dram_tensor("cb", [8, 8, 128, 512], BF16, kind="ExternalInput")
    bq = nc.dram_tensor("bq", [128, FT_Q], F32, kind="ExternalInput")
    bp = nc.dram_tensor("bp", [128, C], F32, kind="ExternalInput")
    ident = nc.dram_tensor("ident", [128, 128], BF16, kind="ExternalInput")
    out = nc.dram_tensor("out", [rows, C], F32, kind="ExternalOutput")

    with TileContext(nc) as tc:
        with tc.tile_pool(name="const", bufs=1) as constp, \
             tc.tile_pool(name="wpool", bufs=1) as wpool, \
             tc.tile_pool(name="stream", bufs=3) as stream, \
             tc.tile_pool(name="acts", bufs=2) as acts, \
             tc.tile_pool(name="otpool", bufs=3) as otpool, \
             tc.tile_pool(name="small", bufs=3) as small, \
             tc.tile_pool(name="pbig", bufs=3, space="PSUM") as pbig, \
             tc.tile_pool(name="pattn", bufs=3, space="PSUM") as pattn, \
             tc.tile_pool(name="ppv", bufs=2, space="PSUM") as ppv:

            # ---- resident constants / weights
            w1_sb = wpool.tile([128, KT_X, 2 * C], FP8)
            nc.sync.dma_start(out=w1_sb, in_=w1.rearrange("(a p) f -> p a f", p=128))
            w2_sb = wpool.tile([128, KT_Y, C], FP8)
            nc.sync.dma_start(out=w2_sb, in_=w2.rearrange("(a p) f -> p a f", p=128))
            wv1_sb = wpool.tile([128, KT_X, C], BF16)
            nc.sync.dma_start(out=wv1_sb, in_=wv1.rearrange("(a p) f -> p a f", p=128))
            wv2_sb = wpool.tile([128, KT_Y, C], BF16)
            nc.sync.dma_start(out=wv2_sb, in_=wv2.rearrange("(a p) f -> p a f", p=128))
            wp_sb = wpool.tile([128, 4, C], BF16)
            nc.sync.dma_start(out=wp_sb, in_=wp.rearrange("a p f -> p a f"))
            bq_sb = constp.tile([128, FT_Q], F32)
            nc.sync.dma_start(out=bq_sb, in_=bq[:, :])
            bp_sb = constp.tile([128, C], F32)
            nc.sync.dma_start(out=bp_sb, in_=bp[:, :])
            # den-matmul stationary: cols 0-31 sum partitions 0-63 (hh=0),
            # cols 32-63 sum partitions 64-127 (hh=1) -- denominator comes
            # out of the PE already replicated across the 32 d-partitions
            ones2_sb = constp.tile([128, 64], BF16)
            nc.vector.memset(ones2_sb[:, :], 0.0)
            nc.vector.memset(ones2_sb[0:64, 0:32], 1.0)
            nc.vector.memset(ones2_sb[64:128, 32:64], 1.0)
            id_sb = constp.tile([128, 128], BF16)
            nc.sync.dma_start(out=id_sb, in_=ident[:, :])

            xt_r = xt.rearrange("(a p) r -> p a r", p=128)
            yt_r = yt.rearrange("(a p) r -> p a r", p=128)
            xtb_r = xtb.rearrange("(a p) r -> p a r", p=128)
            ytb_r = ytb.rearrange("(a p) r -> p a r", p=128)

            st = {}  # per-chunk live tiles

            def emit_dma(c):
                r0 = c * ROWS_PER_CHUNK
                s = {}
                s["xt"] = stream.tile([128, KT_X, ROWS_PER_CHUNK], FP8, tag="xt", name="xt")
                nc.sync.dma_start(out=s["xt"], in_=xt_r[:, :, r0:r0 + ROWS_PER_CHUNK])
                s["yt"] = stream.tile([128, KT_Y, ROWS_PER_CHUNK], FP8, tag="yt", name="yt")
                nc.sync.dma_start(out=s["yt"], in_=yt_r[:, :, r0:r0 + ROWS_PER_CHUNK])
                s["xtb"] = stream.tile([128, KT_X, ROWS_PER_CHUNK], BF16, tag="xtb", name="xtb")
                nc.sync.dma_start(out=s["xtb"], in_=xtb_r[:, :, r0:r0 + ROWS_PER_CHUNK])
                s["ytb"] = stream.tile([128, KT_Y, ROWS_PER_CHUNK], BF16, tag="ytb", name="ytb")
                nc.sync.dma_start(out=s["ytb"], in_=ytb_r[:, :, r0:r0 + ROWS_PER_CHUNK])
                s["cb"] = stream.tile([128, 8, 512], BF16, tag="cb", name="cbt")
                nc.sync.dma_start(out=s["cb"],
                                  in_=cb[c % 8].rearrange("hp p f -> p hp f"))
                s["q"] = acts.tile([128, FT_Q, ROWS_PER_CHUNK], BF16, tag="q", name="qsb")
                s["k"] = acts.tile([128, FT_Q, ROWS_PER_CHUNK], BF16, tag="k", name="ksb")
                # v lives on both partition halves (dup'd by DMA) so PV's
                # stationary base matches the moving exp slice's base 64*hh
                s["v"] = acts.tile([128, WIN_PER_CHUNK, C], BF16, tag="v", name="vsb")
                s["expT"] = acts.tile([128, 8, 512], BF16, tag="expT", name="expT")
                s["ot"] = otpool.tile([128, 4 * ROWS_PER_CHUNK], BF16, tag="ot", name="otsb")
                st[c] = s

            def emit_qkv_group(c, g):
                s = st[c]
                if g < FT_Q:                      # q projection, feature tile g
                    ft = g
                    bank = pbig.tile([128, ROWS_PER_CHUNK], F32, tag="pq")
                    for kt in range(0, KT_X, 2):
                        nc.tensor.matmul(
                            bank[:, :],
                            w1_sb[:, kt:kt + 2, 128 * ft:128 * (ft + 1)],
                            s["xt"][:, kt:kt + 2, :],
                            start=(kt == 0), stop=(kt == KT_X - 2),
                            perf_mode=DR)
                    nc.scalar.activation(
                        s["q"][:, ft, :], bank[:, :],
                        mybir.ActivationFunctionType.Identity,
                        bias=bq_sb[:, ft:ft + 1], scale=1.0 / WS)
                elif g < 2 * FT_Q:                # k projection, feature tile g-4
                    ft = g - FT_Q
                    bank = pbig.tile([128, ROWS_PER_CHUNK], F32, tag="pq")
                    for kt in range(0, KT_X, 2):
                        nc.tensor.matmul(
                            bank[:, :],
                            w1_sb[:, kt:kt + 2, C + 128 * ft:C + 128 * (ft + 1)],
                            s["xt"][:, kt:kt + 2, :],
                            start=(kt == 0), stop=False, perf_mode=DR)
                    for kt in range(0, KT_Y, 2):
                        nc.tensor.matmul(
                            bank[:, :],
                            w2_sb[:, kt:kt + 2, 128 * ft:128 * (ft + 1)],
                            s["yt"][:, kt:kt + 2, :],
                            start=False, stop=(kt == KT_Y - 2), perf_mode=DR)
                    nc.scalar.mul(s["k"][:, ft, :], bank[:, :], 1.0 / WS)
                else:                             # v projection (bf16), row tile g-8
                    rt = g - 2 * FT_Q
                    bank = pbig.tile([128, C], F32, tag="pq")
                    for kt in range(KT_X):
                        nc.tensor.matmul(
                            bank[:, :],
                            s["xtb"][:, kt, 128 * rt:128 * (rt + 1)],
                            wv1_sb[:, kt, :],
                            start=(kt == 0), stop=False)
                    for kt in range(KT_Y):
                        nc.tensor.matmul(
                            bank[:, :],
                            s["ytb"][:, kt, 128 * rt:128 * (rt + 1)],
                            wv2_sb[:, kt, :],
                            start=False, stop=(kt == KT_Y - 1))
                    # v staged into BOTH partition halves (PV stationary for
                    # head-half hh reads partitions 64*hh..64*hh+64)
                    nc.scalar.copy(s["v"][0:64, 2 * rt, :], bank[0:64, :])
                    nc.scalar.copy(s["v"][64:128, 2 * rt, :], bank[0:64, :])
                    nc.vector.tensor_copy(s["v"][0:64, 2 * rt + 1, :], bank[64:128, :])
                    nc.vector.tensor_copy(s["v"][64:128, 2 * rt + 1, :], bank[64:128, :])

            def emit_attn_a(c, hp):
                # QK + bias + exp: the only PE->other-engine handoff; the
                # consuming den/PV matmuls are emitted several slots later
                # (emit_attn_b) so the PE FIFO never stalls on exp.
                s = st[c]
                bank = pattn.tile([128, 512], F32, tag="pattn")
                # combined rpb+mask bias (pre-divided by SCALE, incl -C0
                # shift) written first via identity matmul; the QK matmuls
                # then accumulate onto it (PE-only, no DVE in the chain)
                nc.tensor.matmul(
                    bank[:, :], id_sb[:, :], s["cb"][:, hp, :],
                    start=True, stop=False, skip_group_check=True)
                # S^T[m, n] per (window, head): stationary = k, moving = q
                for sw in range(WIN_PER_CHUNK):
                    for hh in range(2):
                        h = 2 * hp + hh
                        pq = 32 * (h % 4)
                        ft = h // 4
                        nc.tensor.matmul(
                            bank[64 * hh:64 * (hh + 1), 64 * sw:64 * (sw + 1)],
                            s["k"][pq:pq + 32, ft, 64 * sw:64 * (sw + 1)],
                            s["q"][pq:pq + 32, ft, 64 * sw:64 * (sw + 1)],
                            start=False,
                            stop=(sw == WIN_PER_CHUNK - 1 and hh == 1),
                            skip_group_check=True,
                            tile_position=(pq, 64 * hh))
                expT = s["expT"]
                nc.scalar.activation(
                    expT[:, hp, :], bank[:, :],
                    mybir.ActivationFunctionType.Exp, scale=SCALE)

            def emit_attn_b(c, hp):
                s = st[c]
                expT = s["expT"]
                # one bank: den at partitions 0-63 (base 0 --
                # reciprocal_approx_fast misreads partition-offset inputs on
                # HW), unnormalized U at partitions 64-127
                pot = ppv.tile([128, 512], F32, tag="pot")
                nc.tensor.matmul(
                    pot[0:64, :], ones2_sb[:, :], expT[:, hp, :],
                    start=True, stop=True, skip_group_check=True)
                # PV: U[d, n] = sum_m v[m, d] expT[m, n] (unnormalized)
                for sw in range(WIN_PER_CHUNK):
                    for hh in range(2):
                        h = 2 * hp + hh
                        nc.tensor.matmul(
                            pot[64 + 32 * hh:96 + 32 * hh, 64 * sw:64 * (sw + 1)],
                            s["v"][64 * hh:64 * (hh + 1), sw, HD * h:HD * (h + 1)],
                            expT[64 * hh:64 * (hh + 1), hp, 64 * sw:64 * (sw + 1)],
                            start=True, stop=True, skip_group_check=True,
                            tile_position=(64 * hh, 64 + 32 * hh))
                rrep = small.tile([64, 512], F32, tag="rrep")
                nc.vector.reciprocal_approx_fast(rrep[:, :], pot[0:64, :])
                # stage to SBUF with fused normalization:
                # ot[p=32*(h%4)+d, (t, q=h//4, w, n)] = U * (1/den)
                dst = s["ot"][64 * (hp % 2):64 * (hp % 2) + 64, :] \
                    .rearrange("p (t q w m) -> p t q w m", t=4, q=4, w=2) \
                    [:, :, hp // 2, :, :]
                nc.vector.tensor_tensor(
                    out=dst,
                    in0=pot[64:128, :].rearrange("p (t w m) -> p t w m", t=4, w=2),
                    in1=rrep.rearrange("p (t w m) -> p t w m", t=4, w=2),
                    op=mybir.AluOpType.mult)

            def emit_proj_group(c, rt):
                s = st[c]
                r0 = c * ROWS_PER_CHUNK
                bank = pbig.tile([128, C], F32, tag="pq")
                for quad in range(4):
                    nc.tensor.matmul(
                        bank[:, :],
                        s["ot"].rearrange("p (t q f) -> p t q f", t=4, q=4)
                        [:, rt, quad, :],
                        wp_sb[:, quad, :],
                        start=(quad == 0), stop=(quad == 3))
                out_f32 = small.tile([128, C], F32, tag="outf")
                nc.vector.tensor_tensor(out=out_f32[:, :], in0=bank[:, :],
                                        in1=bp_sb[:, :], op=mybir.AluOpType.add)
                nc.sync.dma_start(
                    out=out[r0 + 128 * rt:r0 + 128 * (rt + 1), :],
                    in_=out_f32[:, :])

            # software pipeline: big qkv groups of chunk c interleaved with
            # small attention/proj groups of chunk c-1.  attn part B (den/PV,
            # needs exp of part A) trails part A by two slots so the PE FIFO
            # never waits on the ScalarE exp.
            emit_dma(0)
            for c in range(n_chunks + 1):
                # prefetch next chunk's inputs a full chunk ahead so the
                # chunk-boundary qkv matmuls never wait on DMA
                if c + 1 < n_chunks:
                    emit_dma(c + 1)
                big = [("qkv", c, g) for g in range(12)] if c < n_chunks else []
                smalls = []
                if c > 1:
                    # proj of chunk c-2 leads (all deps satisfied, keeps the
                    # PE fed while chunk c-1's attention chains drain)
                    smalls += [("proj", c - 2, rt) for rt in range(4)]
                if c > 0 and c <= n_chunks:
                    smalls += [("atta", c - 1, 0), ("atta", c - 1, 1)]
                    for hp in range(2, 8):
                        smalls += [("attb", c - 1, hp - 2), ("atta", c - 1, hp)]
                    smalls += [("attb", c - 1, 6), ("attb", c - 1, 7)]
                if c == n_chunks:
                    smalls += [("proj", c - 1, rt) for rt in range(4)]
                order = []
                for i in range(max(len(big), len(smalls))):
                    if i < len(big):
                        order.append(big[i])
                    if i < len(smalls):
                        order.append(smalls[i])
                for kind, cc, idx in order:
                    if kind == "qkv":
                        emit_qkv_group(cc, idx)
                    elif kind == "atta":
                        emit_attn_a(cc, idx)
                    elif kind == "attb":
                        emit_attn_b(cc, idx)
                    else:
                        emit_proj_group(cc, idx)
                if c > 1:
                    del st[c - 2]
    nc.compile()
    return nc


_NC_CACHE = {}


def _get_nc(n_chunks):
    if n_chunks not in _NC_CACHE:
        _NC_CACHE[n_chunks] = build_nc(n_chunks)
    return _NC_CACHE[n_chunks]


def prep_shared(w_qkv1, b_qkv1, w_qkv2, b_qkv2, bias_table, rel_index, w_proj,
                b_proj, mask):
    """Host-side prep of weights/bias tables shared by all cores."""
    # q+k weight columns in fp8 (scaled by WS into e4m3 normal range)
    w1 = np.clip(w_qkv1[:, 0:2 * C] * WS, -240, 240).astype(F8)
    w2k = np.zeros((CYP, C), np.float32)
    w2k[:CY] = w_qkv2[:, C:2 * C]
    # k bias rides an all-ones row in the padded region of yT
    w2k[CY] = b_qkv1[C:2 * C] + b_qkv2[C:2 * C]
    w2 = np.clip(w2k * WS, -240, 240).astype(F8)
    # v weight columns stay bf16
    wv1 = w_qkv1[:, 2 * C:].astype(BF)
    wv2f = np.zeros((CYP, C), np.float32)
    wv2f[:CY] = w_qkv2[:, 2 * C:]
    wv2f[CY] = b_qkv1[2 * C:] + b_qkv2[2 * C:]
    wv2 = wv2f.astype(BF)
    # quad-permuted rows: wp[Q, p, :] = w_proj[32*(4Q + p//32) + p%32, :]
    wp = np.empty((4, 128, C), np.float32)
    for q in range(4):
        for g in range(4):
            wp[q, 32 * g:32 * (g + 1), :] = \
                w_proj[32 * (4 * q + g):32 * (4 * q + g) + 32, :]
    wp = wp.astype(BF)

    bq = b_qkv1[0:C].reshape(FT_Q, 128).T.astype(np.float32).copy()
    bp = np.broadcast_to(b_proj.astype(np.float32), (128, C)).copy()

    rpb = bias_table[rel_index.reshape(-1)].reshape(N, N, H).transpose(2, 0, 1)
    cbt = (rpb[None] + mask[:, None] - C0) / SCALE      # [w, h, n, m]
    cb6 = cbt.reshape(8, 8, 8, 2, N, N)                 # [c8, sw, hp, hh, n, m]
    # transposed bank layout: [c8, hp, (hh, m), (sw, n)]
    cbd = np.ascontiguousarray(cb6.transpose(0, 2, 3, 5, 1, 4)) \
        .reshape(8, 8, 128, 512).astype(BF)

    ident = np.eye(128, dtype=BF)
    return dict(w1=w1, w2=w2, wv1=wv1, wv2=wv2, wp=wp, bq=bq, bp=bp, cb=cbd,
                ident=ident)


def prep_core_inputs(x, y, shared, n_cores=N_CORES):
    """Split x, y along batch, transpose to feature-major, bf16."""
    B_, n, _ = x.shape
    rows = (B_ // n_cores) * n
    in_maps = []
    for i in range(n_cores):
        lo = i * (B_ // n_cores)
        hi = lo + B_ // n_cores
        xs = x[lo:hi].reshape(rows, CX)
        ys = y[lo:hi].reshape(rows, CY)
        xT = np.ascontiguousarray(xs.T)
        yT = np.ascontiguousarray(ys.T)
        xt8 = np.clip(xT, -240, 240).astype(F8)
        yt8 = np.zeros((CYP, rows), F8)
        yt8[:CY] = np.clip(yT, -240, 240).astype(F8)
        yt8[CY] = 1.0
        xtb = xT.astype(BF)
        ytb = np.zeros((CYP, rows), BF)
        ytb[:CY] = yT.astype(BF)
        ytb[CY] = 1.0
        in_maps.append(dict(xt=xt8, yt=yt8, xtb=xtb, ytb=ytb, **shared))
    return in_maps


def kernel(x, y, mask, w_qkv1, b_qkv1, w_qkv2, b_qkv2, bias_table, rel_index,
           w_proj, b_proj, _n_cores=N_CORES, _trace=False):
    B_, n, _ = x.shape
    n_chunks = (B_ // _n_cores) // WIN_PER_CHUNK
    shared = prep_shared(np.asarray(w_qkv1), np.asarray(b_qkv1),
                         np.asarray(w_qkv2), np.asarray(b_qkv2),
                         np.asarray(bias_table), np.asarray(rel_index),
                         np.asarray(w_proj), np.asarray(b_proj),
                         np.asarray(mask))
    in_maps = prep_core_inputs(np.asarray(x), np.asarray(y), shared, _n_cores)
    nc = _get_nc(n_chunks)
    res = run_bass_kernel_spmd(nc, in_maps, core_ids=list(range(_n_cores)),
                               trace=_trace)
    outs = [res.results[i]["out"].reshape(B_ // _n_cores, n, C)
            for i in range(_n_cores)]
    full = np.concatenate(outs, axis=0)
    kernel.last_results = res
    return full


# revision 55
# speedup vs baseline: 1.9168x; 1.0201x over previous
"""Cross-WindowAttention Trainium2 kernel.

Full inputs -> shard batch dim over 8 NeuronCores -> bass/Tile kernel per core
-> gather. Host-side numpy does layout prep (transposes to feature-major,
bf16 conversion, combined rpb+mask bias table); the Bass kernel does all
matmul/softmax compute.

Per-core pipeline (shard = 256 windows of 64 tokens, 16384 rows):
 - qkv projections on PE in bf16, contraction over concat(x,y) for k/v.
   q,k produced feature-major [feat, rows]; v row-major per window [64, 512].
 - attention per (head-pair, 8-window chunk) in one [128, 512] PSUM bank,
   computed TRANSPOSED (S^T[m, n], stationary = k) so the softmax weights
   come out m-major and feed the PV matmul directly -- no PE transposes.
 - softmax normalization is deferred past PV: unnormalized U = exp @ v and
   den = sum_m exp (ones-stationary matmul into the same PSUM bank), then
   U * (1/den) is fused into the PSUM->SBUF staging multiply.  1/den is
   partition-replicated via GpSimd partition_broadcast (idle engine).
 - output projection with attention-output tiles stationary -> row-major
   result, biases via ones-row matmul, contiguous DMA out.

The chunk loop is software-pipelined by one chunk: the small attention/proj
matmul groups of chunk c-1 are emitted interleaved between the large qkv
matmul groups of chunk c, keeping the PE array duty cycle high enough that
the HAM activity monitor does not clock-gate it to half speed.
"""
import numpy as np
import ml_dtypes

import concourse.bacc as bacc
import concourse.mybir as mybir
from concourse.tile import TileContext
from concourse.bass_utils import run_bass_kernel_spmd

F32 = mybir.dt.float32
BF16 = mybir.dt.bfloat16
FP8 = mybir.dt.float8e4
BF = ml_dtypes.bfloat16
F8 = ml_dtypes.float8_e4m3
DR = mybir.MatmulPerfMode.DoubleRow

N_CORES = 8
B_FULL = 2048
N = 64                      # window size (tokens per window)
C = 512                     # channels
H = 16                      # heads
HD = 32                     # head dim
CX = 512                    # x feature dim
CY = 1000                   # y feature dim
CYP = 1024                  # y feature dim padded to multiple of 128
SCALE = HD ** -0.5
C0 = 3.0                    # exp shift: exp(S - C0), cancels in U/den;
                            # keeps fp8 exp outputs within e4m3 range
WS = 64.0                   # fp8 weight scale (into e4m3 normal range)

B_SHARD = B_FULL // N_CORES             # 256 windows per core
WIN_PER_CHUNK = 8
ROWS_PER_CHUNK = WIN_PER_CHUNK * N      # 512
N_CHUNKS = B_SHARD // WIN_PER_CHUNK     # 32

KT_X = CX // 128            # 4 contraction tiles from x
KT_Y = CYP // 128           # 8 contraction tiles from y (padded)
FT_Q = C // 128             # 4 feature tiles per projection output


def build_nc(n_chunks=N_CHUNKS):
    rows = n_chunks * ROWS_PER_CHUNK
    nc = bacc.Bacc("TRN2", target_bir_lowering=False)

    xt = nc.dram_tensor("xt", [CX, rows], FP8, kind="ExternalInput")
    yt = nc.dram_tensor("yt", [CYP, rows], FP8, kind="ExternalInput")
    xtb = nc.dram_tensor("xtb", [CX, rows], BF16, kind="ExternalInput")
    ytb = nc.dram_tensor("ytb", [CYP, rows], BF16, kind="ExternalInput")
    w1 = nc.dram_tensor("w1", [CX, 2 * C], FP8, kind="ExternalInput")
    w2 = nc.dram_tensor("w2", [CYP, C], FP8, kind="ExternalInput")
    wv1 = nc.dram_tensor("wv1", [CX, C], BF16, kind="ExternalInput")
    wv2 = nc.dram_tensor("wv2", [CYP, C], BF16, kind="ExternalInput")
    wp = nc.dram_tensor("wp", [4, 128, C], BF16, kind="ExternalInput")  # quad-permuted rows
    cb = nc.dram_tensor("cb", [8, 8, 128, 512], BF16, kind="ExternalInput")
    bq = nc.dram_tensor("bq", [128, FT_Q], F32, kind="ExternalInput")
    bp = nc.dram_tensor("bp", [128, C], F32, kind="ExternalInput")
    ident = nc.dram_tensor("ident", [128, 128], BF16, kind="ExternalInput")
    out = nc.dram_tensor("out", [rows, C], F32, kind="ExternalOutput")

    with TileContext(nc) as tc:
        with tc.tile_pool(name="const", bufs=1) as constp, \
             tc.tile_pool(name="wpool", bufs=1) as wpool, \
             tc.tile_pool(name="stream", bufs=3) as stream, \
             tc.tile_pool(name="acts", bufs=2) as acts, \
             tc.tile_pool(name="otpool", bufs=3) as otpool, \
             tc.tile_pool(name="small", bufs=3) as small, \
             tc.tile_pool(name="pbig", bufs=3, space="PSUM") as pbig, \
             tc.tile_pool(name="pattn", bufs=3, space="PSUM") as pattn, \
             tc.tile_pool(name="ppv", bufs=2, space="PSUM") as ppv:

            # ---- resident constants / weights
            w1_sb = wpool.tile([128, KT_X, 2 * C], FP8)
            nc.sync.dma_start(out=w1_sb, in_=w1.rearrange("(a p) f -> p a f", p=128))
            w2_sb = wpool.tile([128, KT_Y, C], FP8)
            nc.sync.dma_start(out=w2_sb, in_=w2.rearrange("(a p) f -> p a f", p=128))
            wv1_sb = wpool.tile([128, KT_X, C], BF16)
            nc.sync.dma_start(out=wv1_sb, in_=wv1.rearrange("(a p) f -> p a f", p=128))
            wv2_sb = wpool.tile([128, KT_Y, C], BF16)
            nc.sync.dma_start(out=wv2_sb, in_=wv2.rearrange("(a p) f -> p a f", p=128))
            wp_sb = wpool.tile([128, 4, C], BF16)
            nc.sync.dma_start(out=wp_sb, in_=wp.rearrange("a p f -> p a f"))
            bq_sb = constp.tile([128, FT_Q], F32)
            nc.sync.dma_start(out=bq_sb, in_=bq[:, :])
            bp_sb = constp.tile([128, C], F32)
            nc.sync.dma_start(out=bp_sb, in_=bp[:, :])
            # den-matmul stationary: cols 0-31 sum partitions 0-63 (hh=0),
            # cols 32-63 sum partitions 64-127 (hh=1) -- denominator comes
            # out of the PE already replicated across the 32 d-partitions
            ones2_sb = constp.tile([128, 64], BF16)
            nc.vector.memset(ones2_sb[:, :], 0.0)
            nc.vector.memset(ones2_sb[0:64, 0:32], 1.0)
            nc.vector.memset(ones2_sb[64:128, 32:64], 1.0)
            id_sb = constp.tile([128, 128], BF16)
            nc.sync.dma_start(out=id_sb, in_=ident[:, :])

            xt_r = xt.rearrange("(a p) r -> p a r", p=128)
            yt_r = yt.rearrange("(a p) r -> p a r", p=128)
            xtb_r = xtb.rearrange("(a p) r -> p a r", p=128)
            ytb_r = ytb.rearrange("(a p) r -> p a r", p=128)

            st = {}  # per-chunk live tiles

            def emit_dma(c):
                r0 = c * ROWS_PER_CHUNK
                s = {}
                s["xt"] = stream.tile([128, KT_X, ROWS_PER_CHUNK], FP8, tag="xt", name="xt")
                nc.sync.dma_start(out=s["xt"], in_=xt_r[:, :, r0:r0 + ROWS_PER_CHUNK])
                s["yt"] = stream.tile([128, KT_Y, ROWS_PER_CHUNK], FP8, tag="yt", name="yt")
                nc.sync.dma_start(out=s["yt"], in_=yt_r[:, :, r0:r0 + ROWS_PER_CHUNK])
                s["xtb"] = stream.tile([128, KT_X, ROWS_PER_CHUNK], BF16, tag="xtb", name="xtb")
                nc.sync.dma_start(out=s["xtb"], in_=xtb_r[:, :, r0:r0 + ROWS_PER_CHUNK])
                s["ytb"] = stream.tile([128, KT_Y, ROWS_PER_CHUNK], BF16, tag="ytb", name="ytb")
                nc.sync.dma_start(out=s["ytb"], in_=ytb_r[:, :, r0:r0 + ROWS_PER_CHUNK])
                s["cb"] = stream.tile([128, 8, 512], BF16, tag="cb", name="cbt")
                nc.sync.dma_start(out=s["cb"],
                                  in_=cb[c % 8].rearrange("hp p f -> p hp f"))
                s["q"] = acts.tile([128, FT_Q, ROWS_PER_CHUNK], BF16, tag="q", name="qsb")
                s["k"] = acts.tile([128, FT_Q, ROWS_PER_CHUNK], BF16, tag="k", name="ksb")
                # v lives on both partition halves (dup'd by DMA) so PV's
                # stationary base matches the moving exp slice's base 64*hh
                s["v"] = acts.tile([128, WIN_PER_CHUNK, C], BF16, tag="v", name="vsb")
                s["expT"] = acts.tile([128, 8, 512], BF16, tag="expT", name="expT")
                s["ot"] = otpool.tile([128, 4 * ROWS_PER_CHUNK], BF16, tag="ot", name="otsb")
                st[c] = s

            def emit_qkv_group(c, g):
                s = st[c]
                if g < FT_Q:                      # q projection, feature tile g
                    ft = g
                    bank = pbig.tile([128, ROWS_PER_CHUNK], F32, tag="pq")
                    for kt in range(0, KT_X, 2):
                        nc.tensor.matmul(
                            bank[:, :],
                            w1_sb[:, kt:kt + 2, 128 * ft:128 * (ft + 1)],
                            s["xt"][:, kt:kt + 2, :],
                            start=(kt == 0), stop=(kt == KT_X - 2),
                            perf_mode=DR)
                    nc.scalar.activation(
                        s["q"][:, ft, :], bank[:, :],
                        mybir.ActivationFunctionType.Identity,
                        bias=bq_sb[:, ft:ft + 1], scale=1.0 / WS)
                elif g < 2 * FT_Q:                # k projection, feature tile g-4
                    ft = g - FT_Q
                    bank = pbig.tile([128, ROWS_PER_CHUNK], F32, tag="pq")
                    for kt in range(0, KT_X, 2):
                        nc.tensor.matmul(
                            bank[:, :],
                            w1_sb[:, kt:kt + 2, C + 128 * ft:C + 128 * (ft + 1)],
                            s["xt"][:, kt:kt + 2, :],
                            start=(kt == 0), stop=False, perf_mode=DR)
                    for kt in range(0, KT_Y, 2):
                        nc.tensor.matmul(
                            bank[:, :],
                            w2_sb[:, kt:kt + 2, 128 * ft:128 * (ft + 1)],
                            s["yt"][:, kt:kt + 2, :],
                            start=False, stop=(kt == KT_Y - 2), perf_mode=DR)
                    nc.scalar.mul(s["k"][:, ft, :], bank[:, :], 1.0 / WS)
                else:                             # v projection (bf16), row tile g-8
                    rt = g - 2 * FT_Q
                    bank = pbig.tile([128, C], F32, tag="pq")
                    for kt in range(KT_X):
                        nc.tensor.matmul(
                            bank[:, :],
                            s["xtb"][:, kt, 128 * rt:128 * (rt + 1)],
                            wv1_sb[:, kt, :],
                            start=(kt == 0), stop=False)
                    for kt in range(KT_Y):
                        nc.tensor.matmul(
                            bank[:, :],
                            s["ytb"][:, kt, 128 * rt:128 * (rt + 1)],
                            wv2_sb[:, kt, :],
                            start=False, stop=(kt == KT_Y - 1))
                    # v staged into BOTH partition halves (PV stationary for
                    # head-half hh reads partitions 64*hh..64*hh+64)
                    nc.scalar.copy(s["v"][0:64, 2 * rt, :], bank[0:64, :])
                    nc.scalar.copy(s["v"][64:128, 2 * rt, :], bank[0:64, :])
                    nc.vector.tensor_copy(s["v"][0:64, 2 * rt + 1, :], bank[64:128, :])
                    nc.vector.tensor_copy(s["v"][64:128, 2 * rt + 1, :], bank[64:128, :])

            def emit_attn_a(c, hp):
                # QK + bias + exp: the only PE->other-engine handoff; the
                # consuming den/PV matmuls are emitted several slots later
                # (emit_attn_b) so the PE FIFO never stalls on exp.
                s = st[c]
                bank = pattn.tile([128, 512], F32, tag="pattn")
                # S^T[m, n] per (window, head): stationary = k, moving = q
                for sw in range(WIN_PER_CHUNK):
                    for hh in range(2):
                        h = 2 * hp + hh
                        pq = 32 * (h % 4)
                        ft = h // 4
                        nc.tensor.matmul(
                            bank[64 * hh:64 * (hh + 1), 64 * sw:64 * (sw + 1)],
                            s["k"][pq:pq + 32, ft, 64 * sw:64 * (sw + 1)],
                            s["q"][pq:pq + 32, ft, 64 * sw:64 * (sw + 1)],
                            start=True, stop=True, skip_group_check=True,
                            tile_position=(pq, 64 * hh))
                # combined rpb+mask bias (pre-divided by SCALE, incl -C0
                # shift) added on DVE -- off the PE FIFO's critical path
                # thanks to the A/B split
                nc.vector.tensor_tensor(out=bank[:, :], in0=bank[:, :],
                                        in1=s["cb"][:, hp, :],
                                        op=mybir.AluOpType.add)
                expT = s["expT"]
                nc.scalar.activation(
                    expT[:, hp, :], bank[:, :],
                    mybir.ActivationFunctionType.Exp, scale=SCALE)

            def emit_attn_b(c, hp):
                s = st[c]
                expT = s["expT"]
                # one bank: den at partitions 0-63 (base 0 --
                # reciprocal_approx_fast misreads partition-offset inputs on
                # HW), unnormalized U at partitions 64-127
                pot = ppv.tile([128, 512], F32, tag="pot")
                nc.tensor.matmul(
                    pot[0:64, :], ones2_sb[:, :], expT[:, hp, :],
                    start=True, stop=True, skip_group_check=True)
                # PV: U[d, n] = sum_m v[m, d] expT[m, n] (unnormalized)
                for sw in range(WIN_PER_CHUNK):
                    for hh in range(2):
                        h = 2 * hp + hh
                        nc.tensor.matmul(
                            pot[64 + 32 * hh:96 + 32 * hh, 64 * sw:64 * (sw + 1)],
                            s["v"][64 * hh:64 * (hh + 1), sw, HD * h:HD * (h + 1)],
                            expT[64 * hh:64 * (hh + 1), hp, 64 * sw:64 * (sw + 1)],
                            start=True, stop=True, skip_group_check=True,
                            tile_position=(64 * hh, 64 + 32 * hh))
                rrep = small.tile([64, 512], F32, tag="rrep")
                nc.vector.reciprocal_approx_fast(rrep[:, :], pot[0:64, :])
                # stage to SBUF with fused normalization:
                # ot[p=32*(h%4)+d, (t, q=h//4, w, n)] = U * (1/den)
                dst = s["ot"][64 * (hp % 2):64 * (hp % 2) + 64, :] \
                    .rearrange("p (t q w m) -> p t q w m", t=4, q=4, w=2) \
                    [:, :, hp // 2, :, :]
                nc.vector.tensor_tensor(
                    out=dst,
                    in0=pot[64:128, :].rearrange("p (t w m) -> p t w m", t=4, w=2),
                    in1=rrep.rearrange("p (t w m) -> p t w m", t=4, w=2),
                    op=mybir.AluOpType.mult)

            def emit_proj_group(c, rt):
                s = st[c]
                r0 = c * ROWS_PER_CHUNK
                bank = pbig.tile([128, C], F32, tag="pq")
                for quad in range(4):
                    nc.tensor.matmul(
                        bank[:, :],
                        s["ot"].rearrange("p (t q f) -> p t q f", t=4, q=4)
                        [:, rt, quad, :],
                        wp_sb[:, quad, :],
                        start=(quad == 0), stop=(quad == 3))
                out_f32 = small.tile([128, C], F32, tag="outf")
                nc.vector.tensor_tensor(out=out_f32[:, :], in0=bank[:, :],
                                        in1=bp_sb[:, :], op=mybir.AluOpType.add)
                nc.sync.dma_start(
                    out=out[r0 + 128 * rt:r0 + 128 * (rt + 1), :],
                    in_=out_f32[:, :])

            # software pipeline: big qkv groups of chunk c interleaved with
            # small attention/proj groups of chunk c-1.  attn part B (den/PV,
            # needs exp of part A) trails part A by two slots so the PE FIFO
            # never waits on the ScalarE exp.
            emit_dma(0)
            for c in range(n_chunks + 1):
                # prefetch next chunk's inputs a full chunk ahead so the
                # chunk-boundary qkv matmuls never wait on DMA
                if c + 1 < n_chunks:
                    emit_dma(c + 1)
                big = [("qkv", c, g) for g in range(12)] if c < n_chunks else []
                smalls = []
                if c > 1:
                    # proj of chunk c-2 leads (all deps satisfied, keeps the
                    # PE fed while chunk c-1's attention chains drain)
                    smalls += [("proj", c - 2, rt) for rt in range(4)]
                if c > 0 and c <= n_chunks:
                    smalls += [("atta", c - 1, 0), ("atta", c - 1, 1)]
                    for hp in range(2, 8):
                        smalls += [("attb", c - 1, hp - 2), ("atta", c - 1, hp)]
                    smalls += [("attb", c - 1, 6), ("attb", c - 1, 7)]
                if c == n_chunks:
                    smalls += [("proj", c - 1, rt) for rt in range(4)]
                order = []
                for i in range(max(len(big), len(smalls))):
                    if i < len(big):
                        order.append(big[i])
                    if i < len(smalls):
                        order.append(smalls[i])
                for kind, cc, idx in order:
                    if kind == "qkv":
                        emit_qkv_group(cc, idx)
                    elif kind == "atta":
                        emit_attn_a(cc, idx)
                    elif kind == "attb":
                        emit_attn_b(cc, idx)
                    else:
                        emit_proj_group(cc, idx)
                if c > 1:
                    del st[c - 2]
    nc.compile()
    return nc


_NC_CACHE = {}


def _get_nc(n_chunks):
    if n_chunks not in _NC_CACHE:
        _NC_CACHE[n_chunks] = build_nc(n_chunks)
    return _NC_CACHE[n_chunks]


def prep_shared(w_qkv1, b_qkv1, w_qkv2, b_qkv2, bias_table, rel_index, w_proj,
                b_proj, mask):
    """Host-side prep of weights/bias tables shared by all cores."""
    # q+k weight columns in fp8 (scaled by WS into e4m3 normal range)
    w1 = np.clip(w_qkv1[:, 0:2 * C] * WS, -240, 240).astype(F8)
    w2k = np.zeros((CYP, C), np.float32)
    w2k[:CY] = w_qkv2[:, C:2 * C]
    # k bias rides an all-ones row in the padded region of yT
    w2k[CY] = b_qkv1[C:2 * C] + b_qkv2[C:2 * C]
    w2 = np.clip(w2k * WS, -240, 240).astype(F8)
    # v weight columns stay bf16
    wv1 = w_qkv1[:, 2 * C:].astype(BF)
    wv2f = np.zeros((CYP, C), np.float32)
    wv2f[:CY] = w_qkv2[:, 2 * C:]
    wv2f[CY] = b_qkv1[2 * C:] + b_qkv2[2 * C:]
    wv2 = wv2f.astype(BF)
    # quad-permuted rows: wp[Q, p, :] = w_proj[32*(4Q + p//32) + p%32, :]
    wp = np.empty((4, 128, C), np.float32)
    for q in range(4):
        for g in range(4):
            wp[q, 32 * g:32 * (g + 1), :] = \
                w_proj[32 * (4 * q + g):32 * (4 * q + g) + 32, :]
    wp = wp.astype(BF)

    bq = b_qkv1[0:C].reshape(FT_Q, 128).T.astype(np.float32).copy()
    bp = np.broadcast_to(b_proj.astype(np.float32), (128, C)).copy()

    rpb = bias_table[rel_index.reshape(-1)].reshape(N, N, H).transpose(2, 0, 1)
    cbt = (rpb[None] + mask[:, None] - C0) / SCALE      # [w, h, n, m]
    cb6 = cbt.reshape(8, 8, 8, 2, N, N)                 # [c8, sw, hp, hh, n, m]
    # transposed bank layout: [c8, hp, (hh, m), (sw, n)]
    cbd = np.ascontiguousarray(cb6.transpose(0, 2, 3, 5, 1, 4)) \
        .reshape(8, 8, 128, 512).astype(BF)

    ident = np.eye(128, dtype=BF)
    return dict(w1=w1, w2=w2, wv1=wv1, wv2=wv2, wp=wp, bq=bq, bp=bp, cb=cbd,
                ident=ident)


def prep_core_inputs(x, y, shared, n_cores=N_CORES):
    """Split x, y along batch, transpose to feature-major, bf16."""
    B_, n, _ = x.shape
    rows = (B_ // n_cores) * n
    in_maps = []
    for i in range(n_cores):
        lo = i * (B_ // n_cores)
        hi = lo + B_ // n_cores
        xs = x[lo:hi].reshape(rows, CX)
        ys = y[lo:hi].reshape(rows, CY)
        xT = np.ascontiguousarray(xs.T)
        yT = np.ascontiguousarray(ys.T)
        xt8 = np.clip(xT, -240, 240).astype(F8)
        yt8 = np.zeros((CYP, rows), F8)
        yt8[:CY] = np.clip(yT, -240, 240).astype(F8)
        yt8[CY] = 1.0
        xtb = xT.astype(BF)
        ytb = np.zeros((CYP, rows), BF)
        ytb[:CY] = yT.astype(BF)
        ytb[CY] = 1.0
        in_maps.append(dict(xt=xt8, yt=yt8, xtb=xtb, ytb=ytb, **shared))
    return in_maps


def kernel(x, y, mask, w_qkv1, b_qkv1, w_qkv2, b_qkv2, bias_table, rel_index,
           w_proj, b_proj, _n_cores=N_CORES, _trace=False):
    B_, n, _ = x.shape
    n_chunks = (B_ // _n_cores) // WIN_PER_CHUNK
    shared = prep_shared(np.asarray(w_qkv1), np.asarray(b_qkv1),
                         np.asarray(w_qkv2), np.asarray(b_qkv2),
                         np.asarray(bias_table), np.asarray(rel_index),
                         np.asarray(w_proj), np.asarray(b_proj),
                         np.asarray(mask))
    in_maps = prep_core_inputs(np.asarray(x), np.asarray(y), shared, _n_cores)
    nc = _get_nc(n_chunks)
    res = run_bass_kernel_spmd(nc, in_maps, core_ids=list(range(_n_cores)),
                               trace=_trace)
    outs = [res.results[i]["out"].reshape(B_ // _n_cores, n, C)
            for i in range(_n_cores)]
    full = np.concatenate(outs, axis=0)
    kernel.last_results = res
    return full
